# revision 12
# baseline (speedup 1.0000x reference)
"""Causal self-attention (B=4, T=2048, C=1024, H=16, Dh=64) on 8 trn2 NeuronCores.

Sharding: core = 2*b + g  (b = batch 0..3, g = head-group 0..1, 8 heads each).
Each core computes its batch's QKV projection for its 8 heads, causal
attention, and a partial out-projection; host sums the two head-group
partials per batch (the tensor-parallel "all-reduce").

v3 design (per core), single software-pipelined loop:
  - All projections bf16 (fp8 there costs ~3-5% output error).
  - q^T/k^T bf16 [j, t]; S^T[tk, tq] per head-pair computed into fp32-psum
    chunks [128, 2, 512] (both heads) -> ONE fused exp (ACT) per tk tile.
  - exp writes P: diagonal-straddling tiles -> bf16; strictly-causal tiles ->
    fp8 e5m2 (no max-subtraction; e5m2 spans e^-14..e^11; softmax averaging
    damps the 2-bit mantissa noise).
  - PV: off-diag via fp8 DoubleRow over tk-tile pairs (V e4m3 lhsT, d-dim
    padded to 80 for the 16B DoubleRow stride rule, ones col 64 = rowsum);
    diag tiles bf16. PV emission lags exp by one chunk so the PE never
    blocks on ACT.
  - Projection / V / out-projection matmul chunks are interleaved between
    attention chunks from an ordered filler queue, keeping the PE
    continuously busy (full 2.4 GHz pstate) across the whole kernel.
  - reciprocal + K=1 ones matmul broadcasts 1/rowsum; DVE mul -> y^T bf16.
  - out-projection bf16, one tq-block behind attention.
"""

import sys

for _p in ("/opt/trn_rl_repo", "/opt/pypackages"):
    if _p not in sys.path:
        sys.path.append(_p)

import numpy as np
import ml_dtypes
from contextlib import ExitStack

import concourse.bass as bass
import concourse.tile as tile
from concourse import bacc, mybir
from concourse.bass_utils import run_bass_kernel_spmd

B, T, C = 4, 2048, 1024
H, DH = 16, 64
HG = 8          # heads per core
JW = 512        # tq tile width
NT = T // JW    # 4 tq tiles
NK = T // 128   # 16 tk tiles
NC_ = C // 128  # 8 contraction tiles
VP = 80         # padded V free dim (16B-aligned for DoubleRow)
MASK_VAL = -1.0e8
F32 = mybir.dt.float32
F32R = mybir.dt.float32r
BF16 = mybir.dt.bfloat16
FP8E4 = mybir.dt.float8e4
FP8E5 = mybir.dt.float8e5
EXP = mybir.ActivationFunctionType.Exp
DR = mybir.MatmulPerfMode.DoubleRow

E4NP = ml_dtypes.float8_e4m3
E5NP = ml_dtypes.float8_e5m2
BFNP = ml_dtypes.bfloat16

_cache = {}


def _build():
    nc = bacc.Bacc("TRN2", target_bir_lowering=False, debug=False, num_devices=8)
    xtb_d = nc.dram_tensor("xtb", [128, NC_ * T], BF16, kind="ExternalInput").ap()
    wqk_d = nc.dram_tensor("wqk", [128, NC_ * 1024], BF16, kind="ExternalInput").ap()
    wv_d = nc.dram_tensor("wv", [128, NC_ * 512], BF16, kind="ExternalInput").ap()
    wout_d = nc.dram_tensor("wout", [512, C], BF16, kind="ExternalInput").ap()
    dmask_d = nc.dram_tensor("dmask", [128, 128], F32, kind="ExternalInput").ap()
    ones_row = nc.dram_tensor("ones_row", [1, 64], F32R, kind="ExternalInput").ap()
    out = nc.dram_tensor("out", [T, C], F32, kind="ExternalOutput").ap()

    with tile.TileContext(nc) as tc:
        with ExitStack() as ctx:
            ctx.enter_context(nc.allow_low_precision(reason="fp8/bf16 mixed precision intended"))
            # ---- persistent SBUF ----
            big = ctx.enter_context(tc.tile_pool(name="big", bufs=1))
            qk_sb = [big.tile([128, T], BF16, tag=f"qk{j}", name=f"qk_sb{j}") for j in range(8)]
            v8 = big.tile([128, 8 * HG * 2 * VP], FP8E4, tag="v8", name="v8")
            v8v = v8[:].rearrange("p (pr h t d) -> p pr h t d", pr=8, h=HG, t=2, d=VP)
            vb = big.tile([128, NK * HG * 65], BF16, tag="vb", name="vb")
            vbv = vb[:].rearrange("p (i h d) -> p i h d", i=NK, h=HG, d=65)
            y_sb = [big.tile([128, T], BF16, tag=f"y{m}", name=f"y_sb{m}") for m in range(4)]
            onesr = big.tile([1, 64], F32R, tag="onesr", name="onesr")
            dmask_sb = big.tile([128, 128], F32, tag="dm", name="dmask_sb")
            xtb = big.tile([128, NC_ * T], BF16, tag="xtb", name="xtb")
            wqkb = big.tile([128, NC_ * 1024], BF16, tag="wqkb", name="wqkb")
            wvb = big.tile([128, NC_ * 512], BF16, tag="wvb", name="wvb")
            wo_sb = big.tile([128, 2 * 4 * 512], BF16, tag="wo", name="wo_sb")
            wov = wo_sb[:].rearrange("p (e j) -> p e j", e=2, j=4 * 512)

            nc.gpsimd.dma_start(onesr[:], ones_row[:])
            nc.gpsimd.dma_start(dmask_sb[:], dmask_d[:])
            xtv = xtb[:].rearrange("p (c t) -> p c t", c=NC_, t=T)
            # xtb DMA split by tq-slice so the first projection can start early
            for tt in range(NT):
                nc.gpsimd.dma_start(xtv[:, :, JW * tt:JW * tt + JW],
                                    xtb_d[:].rearrange("p (c t) -> p c t", c=NC_, t=T)
                                    [:, :, JW * tt:JW * tt + JW])
            nc.gpsimd.dma_start(wqkb[:], wqk_d[:])
            nc.gpsimd.dma_start(wvb[:], wv_d[:])
            for jt in range(4):
                for et in range(2):
                    nc.gpsimd.dma_start(
                        wov[:, et, 512 * jt:512 * jt + 512],
                        wout_d[128 * jt:128 * jt + 128, 512 * et:512 * et + 512])
            wqkv = wqkb[:].rearrange("p (c j) -> p c j", c=NC_, j=1024)
            wvv = wvb[:].rearrange("p (c j) -> p c j", c=NC_, j=512)
            for pr in range(8):
                nc.vector.memset(v8v[:, pr, :, :, 64], 1.0)
                nc.vector.memset(v8v[:, pr, :, :, 65:VP], 0.0)
            nc.vector.memset(vbv[:, :, :, 64], 1.0)

            # P buffers
            p8_pool = ctx.enter_context(tc.tile_pool(name="p8", bufs=2))
            pd_pool = ctx.enter_context(tc.tile_pool(name="pd", bufs=2))
            fin_pool = ctx.enter_context(tc.tile_pool(name="fin", bufs=2))
            # PSUM: s 2x2 banks + ya/yb 1 each + aux 2 = 8 banks
            s_psum = ctx.enter_context(tc.tile_pool(name="s_psum", bufs=2, space="PSUM"))
            y_psum = ctx.enter_context(tc.tile_pool(name="y_psum", bufs=1, space="PSUM"))
            aux_psum = ctx.enter_context(tc.tile_pool(name="aux_psum", bufs=2, space="PSUM"))
            o_pool = ctx.enter_context(tc.tile_pool(name="o", bufs=2))

            # ---------- PE work-unit emitters ----------
            def proj_qk(jt, tt):
                def emit():
                    ps = aux_psum.tile([128, JW], F32, tag="aux", name="psaux")
                    for ct in range(NC_):
                        nc.tensor.matmul(
                            ps[:], wqkv[:, ct, 128 * jt:128 * jt + 128],
                            xtv[:, ct, JW * tt:JW * tt + JW],
                            start=(ct == 0), stop=(ct == NC_ - 1))
                    nc.vector.tensor_copy(qk_sb[jt][:, JW * tt:JW * tt + JW], ps[:])
                return emit

            def proj_v(it):
                def emit():
                    ps = aux_psum.tile([128, JW], F32, tag="aux", name="psaux")
                    for ct in range(NC_):
                        nc.tensor.matmul(
                            ps[:], xtv[:, ct, 128 * it:128 * it + 128],
                            wvv[:, ct, :],
                            start=(ct == 0), stop=(ct == NC_ - 1))
                    psv = ps[:].rearrange("p (h d) -> p h d", h=HG, d=64)
                    nc.vector.tensor_copy(v8v[:, it // 2, :, it % 2, 0:64], psv)
                    nc.vector.tensor_copy(vbv[:, it, :, 0:64], psv)
                return emit

            def outproj(it, et):
                def emit():
                    ps = aux_psum.tile([128, JW], F32, tag="aux", name="psaux")
                    for jt in range(4):
                        nc.tensor.matmul(
                            ps[:], y_sb[jt][:, 128 * it:128 * it + 128],
                            wov[:, et, 512 * jt:512 * jt + 512],
                            start=(jt == 0), stop=(jt == 3))
                    ot = o_pool.tile([128, 512], F32, tag="ot", name="ot")
                    nc.vector.tensor_copy(ot[:], ps[:])
                    nc.sync.dma_start(
                        out[128 * it:128 * it + 128, 512 * et:512 * et + 512], ot[:])
                return emit

            # ordered filler queue with availability gating
            fillers = []          # list of closures
            ready = []            # parallel list of bools
            drained = [0]         # next index to drain

            def add_fill(fn, is_ready=True):
                fillers.append(fn)
                ready.append(is_ready)
                return len(fillers) - 1

            def drain(n):
                k = 0
                while k < n and drained[0] < len(fillers) and ready[drained[0]]:
                    fillers[drained[0]]()
                    drained[0] += 1
                    k += 1

            def flush_to(idx):
                while drained[0] < idx:
                    assert ready[drained[0]], f"filler {drained[0]} not ready"
                    fillers[drained[0]]()
                    drained[0] += 1

            # build the static filler order
            levels = {}
            for m in range(1, 4):
                add_fill(proj_qk(4 + m, 0))
                add_fill(proj_qk(m, 0))
                levels[(0, m)] = len(fillers)
            op_idx = {}
            for J in range(1, NT):
                for it in range(4 * J, 4 * J + 4):
                    add_fill(proj_v(it))
                for jt in (4, 5, 6, 7, 0, 1, 2, 3):
                    add_fill(proj_qk(jt, J))
                levels[(J, 0)] = len(fillers)
                # out-projection of block J-1 (gated on normalize of J-1)
                ops = []
                for it in range(4 * (J - 1), 4 * (J - 1) + 4):
                    for et in range(2):
                        ops.append(add_fill(outproj(it, et), is_ready=False))
                op_idx[J - 1] = ops
            ops = []
            for it in range(4 * 3, 4 * 3 + 4):
                for et in range(2):
                    ops.append(add_fill(outproj(it, et), is_ready=False))
            op_idx[3] = ops

            # ---------- phase 0: first projections ----------
            proj_qk(4, 0)()   # k of head-pair 0
            proj_qk(0, 0)()   # q of head-pair 0
            for it in range(4):
                proj_v(it)()

            # ---------- main attention loop ----------
            ESC = 0.125
            pending_norm = []
            for J in range(NT):
                pace = 1 if J <= 1 else (2 if J == 2 else 8)
                for m in range(4):
                    lvl = levels.get((J, m))
                    if lvl is not None:
                        flush_to(lvl)
                    psy = {0: y_psum.tile([VP, JW], F32, tag="ya", name="psya"),
                           64: y_psum.tile([VP, JW], F32, tag="yb", name="psyb")}
                    nki = 4 * J + 4
                    p8v = None
                    if J > 0:
                        p8 = p8_pool.tile([128, 2 * 6 * 2 * JW], FP8E5, tag="p8")
                        p8v = p8[:].rearrange("p (o pr t q) -> p o pr t q",
                                              o=2, pr=6, t=2, q=JW)
                    pd = pd_pool.tile([128, 2 * 4 * JW], BF16, tag="pd")
                    pdv = pd[:].rearrange("p (o r q) -> p o r q", o=2, r=4, q=JW)

                    first_pv = {0: True, 64: True}
                    n_pv = (2 * J) + 4          # DR pairs + diag singles per off
                    pv_done = {0: 0, 64: 0}
                    pending_pv = []

                    def emit_pv():
                        for fn in pending_pv:
                            fn()
                        pending_pv.clear()

                    for i in range(nki):
                        r = i - 4 * J
                        lo = 128 * r if r > 0 else 0
                        sch = s_psum.tile([128, 2 * JW], F32, tag="s", name="S")
                        schv = sch[:].rearrange("p (o q) -> p o q", o=2, q=JW)
                        for oi, off in enumerate((0, 64)):
                            nc.tensor.matmul(
                                schv[:, oi, lo:JW],
                                qk_sb[4 + m][off:off + 64, 128 * i:128 * i + 128],
                                qk_sb[m][off:off + 64, JW * J + lo:JW * J + JW],
                                start=True, stop=True)
                        if r >= 0:
                            for oi in range(2):
                                nc.vector.tensor_add(
                                    schv[:, oi, 128 * r:128 * r + 128],
                                    schv[:, oi, 128 * r:128 * r + 128],
                                    dmask_sb[:])
                            nc.scalar.activation(
                                pdv[:, :, r, lo:JW], schv[:, :, lo:JW], EXP, scale=ESC)

                            def mk_diag(i=i, r=r, lo=lo):
                                def go():
                                    for oi, off in enumerate((0, 64)):
                                        h = 2 * m + oi
                                        pv_done[off] += 1
                                        nc.tensor.matmul(
                                            psy[off][0:65, lo:JW],
                                            vbv[:, i, h, :],
                                            pdv[:, oi, r, lo:JW],
                                            start=first_pv[off],
                                            stop=(pv_done[off] == n_pv),
                                            skip_group_check=True)
                                        first_pv[off] = False
                                return go
                            pending_pv.append(mk_diag())
                        else:
                            nc.scalar.activation(
                                p8v[:, :, i // 2, i % 2, :], schv[:, :, :], EXP, scale=ESC)
                            if i % 2 == 1:
                                def mk_pair(i=i):
                                    def go():
                                        for oi, off in enumerate((0, 64)):
                                            h = 2 * m + oi
                                            pv_done[off] += 1
                                            nc.tensor.matmul(
                                                psy[off][:],
                                                v8v[:, i // 2, h, :, :],
                                                p8v[:, oi, i // 2, :, :],
                                                start=first_pv[off],
                                                stop=(pv_done[off] == n_pv),
                                                perf_mode=DR, skip_group_check=True)
                                            first_pv[off] = False
                                    return go
                                pending_pv.append(mk_pair())
                        # filler work covers the exp latency before PV lands
                        if i % pace == 0:
                            drain(1)
                        if i == 1 and pending_norm:
                            for fn in pending_norm:
                                fn()
                            pending_norm.clear()
                        if i >= 1:
                            emit_pv()
                    emit_pv()

                    def mk_norm(m=m, J=J, psy=psy):
                        def go():
                            for off in (0, 64):
                                rsr = fin_pool.tile([1, JW], F32, tag="rsr", name="rsr")
                                nc.vector.tensor_copy(rsr[:], psy[off][64:65, :])
                                rrow = fin_pool.tile([1, JW], F32, tag="rrow", name="rrow")
                                nc.vector.reciprocal_approx_fast(rrow[:], rsr[:])
                                rec = fin_pool.tile([64, JW], F32, tag="rec", name="rec")
                                nc.gpsimd.partition_broadcast(rec[:], rrow[:])
                                nc.vector.tensor_mul(
                                    y_sb[m][off:off + 64, JW * J:JW * J + JW],
                                    psy[off][0:64, :], rec[:])
                            if m == 3:
                                # y(J) now complete: release its out-proj fillers
                                for idx in op_idx[J]:
                                    ready[idx] = True
                        return go
                    pending_norm.append(mk_norm())
                # last block: no next chunk loop to host the deferred normalize
                if J == NT - 1:
                    for fn in pending_norm:
                        fn()
                    pending_norm.clear()
            # tail: remaining out-projection (+ any stragglers)
            flush_to(len(fillers))
    nc.compile()
    return nc


def _host_masks():
    a = np.arange(128, dtype=np.int64)[:, None]
    b = np.arange(128, dtype=np.int64)[None, :]
    return np.where(a <= b, np.float32(0.0), np.float32(MASK_VAL))


def _pack_ct(arr):
    """[1024, n] f32 -> [128, 8*n] bf16 with c = 128*ct + p packing."""
    n = arr.shape[1]
    return np.ascontiguousarray(
        arr.reshape(NC_, 128, n).transpose(1, 0, 2).reshape(128, NC_ * n)
        .astype(BFNP))


def _make_in_map(core, x, w_qkv, w_out):
    b, g = divmod(core, 2)
    xT = np.ascontiguousarray(x[b].T)
    wqk = np.concatenate(
        [w_qkv[:, 512 * g:512 * g + 512],
         w_qkv[:, 1024 + 512 * g:1024 + 512 * g + 512]], axis=1)
    wv = w_qkv[:, 2048 + 512 * g:2048 + 512 * g + 512]
    wout_s = np.ascontiguousarray(w_out[512 * g:512 * g + 512, :]).astype(BFNP)
    return dict(
        xtb=_pack_ct(xT),
        wqk=_pack_ct(wqk),
        wv=_pack_ct(wv),
        wout=wout_s,
        dmask=_host_masks(),
        ones_row=np.ones((1, 64), np.float32))


def kernel(x, w_qkv, w_out):
    x = np.ascontiguousarray(x, dtype=np.float32)
    w_qkv = np.ascontiguousarray(w_qkv, dtype=np.float32)
    w_out = np.ascontiguousarray(w_out, dtype=np.float32)

    if "nc" not in _cache:
        _cache["nc"] = _build()
    nc = _cache["nc"]

    in_maps = [_make_in_map(core, x, w_qkv, w_out) for core in range(8)]

    res = run_bass_kernel_spmd(nc, in_maps, core_ids=list(range(8)))
    out = np.empty((B, T, C), np.float32)
    for b in range(B):
        out[b] = res.results[2 * b]["out"] + res.results[2 * b + 1]["out"]
    return out


# revision 13
# speedup vs baseline: 1.1929x; 1.1929x over previous
"""Causal self-attention (B=4, T=2048, C=1024, H=16, Dh=64) on 8 trn2 NeuronCores.

Sharding: core = 2*b + g  (b = batch 0..3, g = head-group 0..1, 8 heads each).
Each core computes its batch's QKV projection for its 8 heads, causal
attention, and a partial out-projection; host sums the two head-group
partials per batch (the tensor-parallel "all-reduce").

v3 design (per core), single software-pipelined loop:
  - All projections bf16 (fp8 there costs ~3-5% output error).
  - q^T/k^T bf16 [j, t]; S^T[tk, tq] per head-pair computed into fp32-psum
    chunks [128, 2, 512] (both heads) -> ONE fused exp (ACT) per tk tile.
  - exp writes P: diagonal-straddling tiles -> bf16; strictly-causal tiles ->
    fp8 e5m2 (no max-subtraction; e5m2 spans e^-14..e^11; softmax averaging
    damps the 2-bit mantissa noise).
  - PV: off-diag via fp8 DoubleRow over tk-tile pairs (V e4m3 lhsT, d-dim
    padded to 80 for the 16B DoubleRow stride rule, ones col 64 = rowsum);
    diag tiles bf16. PV emission lags exp by one chunk so the PE never
    blocks on ACT.
  - Projection / V / out-projection matmul chunks are interleaved between
    attention chunks from an ordered filler queue, keeping the PE
    continuously busy (full 2.4 GHz pstate) across the whole kernel.
  - reciprocal + K=1 ones matmul broadcasts 1/rowsum; DVE mul -> y^T bf16.
  - out-projection bf16, one tq-block behind attention.
"""

import sys

for _p in ("/opt/trn_rl_repo", "/opt/pypackages"):
    if _p not in sys.path:
        sys.path.append(_p)

import numpy as np
import ml_dtypes
from contextlib import ExitStack

import concourse.bass as bass
import concourse.tile as tile
from concourse import bacc, mybir
from concourse.bass_utils import run_bass_kernel_spmd

B, T, C = 4, 2048, 1024
H, DH = 16, 64
HG = 8          # heads per core
JW = 512        # tq tile width
NT = T // JW    # 4 tq tiles
NK = T // 128   # 16 tk tiles
NC_ = C // 128  # 8 contraction tiles
VP = 80         # padded V free dim (16B-aligned for DoubleRow)
MASK_VAL = -1.0e8
F32 = mybir.dt.float32
F32R = mybir.dt.float32r
BF16 = mybir.dt.bfloat16
FP8E4 = mybir.dt.float8e4
FP8E5 = mybir.dt.float8e5
EXP = mybir.ActivationFunctionType.Exp
DR = mybir.MatmulPerfMode.DoubleRow

E4NP = ml_dtypes.float8_e4m3
E5NP = ml_dtypes.float8_e5m2
BFNP = ml_dtypes.bfloat16

_cache = {}


def _build():
    nc = bacc.Bacc("TRN2", target_bir_lowering=False, debug=False, num_devices=8)
    xtb_d = nc.dram_tensor("xtb", [128, NC_ * T], BF16, kind="ExternalInput").ap()
    wqk_d = nc.dram_tensor("wqk", [128, NC_ * 1024], BF16, kind="ExternalInput").ap()
    wv_d = nc.dram_tensor("wv", [128, NC_ * 512], BF16, kind="ExternalInput").ap()
    wout_d = nc.dram_tensor("wout", [512, C], BF16, kind="ExternalInput").ap()
    dmask_d = nc.dram_tensor("dmask", [128, 128], F32, kind="ExternalInput").ap()
    ones_row = nc.dram_tensor("ones_row", [1, 64], F32R, kind="ExternalInput").ap()
    out = nc.dram_tensor("out", [T, C], F32, kind="ExternalOutput").ap()

    with tile.TileContext(nc) as tc:
        with ExitStack() as ctx:
            ctx.enter_context(nc.allow_low_precision(reason="fp8/bf16 mixed precision intended"))
            # ---- persistent SBUF ----
            big = ctx.enter_context(tc.tile_pool(name="big", bufs=1))
            qk_sb = [big.tile([128, T], BF16, tag=f"qk{j}", name=f"qk_sb{j}") for j in range(8)]
            v8 = big.tile([128, 8 * HG * 2 * VP], FP8E4, tag="v8", name="v8")
            v8v = v8[:].rearrange("p (pr h t d) -> p pr h t d", pr=8, h=HG, t=2, d=VP)
            vb = big.tile([128, NK * HG * 65], BF16, tag="vb", name="vb")
            vbv = vb[:].rearrange("p (i h d) -> p i h d", i=NK, h=HG, d=65)
            y_sb = [big.tile([128, T], BF16, tag=f"y{m}", name=f"y_sb{m}") for m in range(4)]
            onesr = big.tile([1, 64], F32R, tag="onesr", name="onesr")
            dmask_sb = big.tile([128, 128], F32, tag="dm", name="dmask_sb")
            xtb = big.tile([128, NC_ * T], BF16, tag="xtb", name="xtb")
            wqkb = big.tile([128, NC_ * 1024], BF16, tag="wqkb", name="wqkb")
            wvb = big.tile([128, NC_ * 512], BF16, tag="wvb", name="wvb")
            wo_sb = big.tile([128, 2 * 4 * 512], BF16, tag="wo", name="wo_sb")
            wov = wo_sb[:].rearrange("p (e j) -> p e j", e=2, j=4 * 512)

            nc.gpsimd.dma_start(onesr[:], ones_row[:])
            nc.gpsimd.dma_start(dmask_sb[:], dmask_d[:])
            xtv = xtb[:].rearrange("p (c t) -> p c t", c=NC_, t=T)
            # xtb DMA split by tq-slice so the first projection can start early
            for tt in range(NT):
                nc.gpsimd.dma_start(xtv[:, :, JW * tt:JW * tt + JW],
                                    xtb_d[:].rearrange("p (c t) -> p c t", c=NC_, t=T)
                                    [:, :, JW * tt:JW * tt + JW])
            nc.gpsimd.dma_start(wqkb[:], wqk_d[:])
            nc.gpsimd.dma_start(wvb[:], wv_d[:])
            for jt in range(4):
                for et in range(2):
                    nc.gpsimd.dma_start(
                        wov[:, et, 512 * jt:512 * jt + 512],
                        wout_d[128 * jt:128 * jt + 128, 512 * et:512 * et + 512])
            wqkv = wqkb[:].rearrange("p (c j) -> p c j", c=NC_, j=1024)
            wvv = wvb[:].rearrange("p (c j) -> p c j", c=NC_, j=512)
            for pr in range(8):
                nc.vector.memset(v8v[:, pr, :, :, 64], 1.0)
                nc.vector.memset(v8v[:, pr, :, :, 65:VP], 0.0)
            nc.vector.memset(vbv[:, :, :, 64], 1.0)

            # P buffers
            p8_pool = ctx.enter_context(tc.tile_pool(name="p8", bufs=2))
            pd_pool = ctx.enter_context(tc.tile_pool(name="pd", bufs=2))
            fin_pool = ctx.enter_context(tc.tile_pool(name="fin", bufs=2))
            # PSUM: s 2x2 banks + ya/yb 1 each + aux 2 = 8 banks
            s_psum = ctx.enter_context(tc.tile_pool(name="s_psum", bufs=2, space="PSUM"))
            y_psum = ctx.enter_context(tc.tile_pool(name="y_psum", bufs=1, space="PSUM"))
            aux_psum = ctx.enter_context(tc.tile_pool(name="aux_psum", bufs=2, space="PSUM"))
            o_pool = ctx.enter_context(tc.tile_pool(name="o", bufs=2))

            # ---------- PE work-unit emitters ----------
            def proj_qk(jt, tt):
                def emit():
                    ps = aux_psum.tile([128, JW], F32, tag="aux", name="psaux")
                    for ct in range(NC_):
                        nc.tensor.matmul(
                            ps[:], wqkv[:, ct, 128 * jt:128 * jt + 128],
                            xtv[:, ct, JW * tt:JW * tt + JW],
                            start=(ct == 0), stop=(ct == NC_ - 1))
                    nc.scalar.copy(qk_sb[jt][:, JW * tt:JW * tt + JW], ps[:])
                return emit

            def proj_v(it):
                def emit():
                    ps = aux_psum.tile([128, JW], F32, tag="aux", name="psaux")
                    for ct in range(NC_):
                        nc.tensor.matmul(
                            ps[:], xtv[:, ct, 128 * it:128 * it + 128],
                            wvv[:, ct, :],
                            start=(ct == 0), stop=(ct == NC_ - 1))
                    psv = ps[:].rearrange("p (h d) -> p h d", h=HG, d=64)
                    nc.scalar.copy(v8v[:, it // 2, :, it % 2, 0:64], psv)
                    nc.vector.tensor_copy(vbv[:, it, :, 0:64], psv)
                return emit

            def outproj(it, et):
                def emit():
                    ps = aux_psum.tile([128, JW], F32, tag="aux", name="psaux")
                    for jt in range(4):
                        nc.tensor.matmul(
                            ps[:], y_sb[jt][:, 128 * it:128 * it + 128],
                            wov[:, et, 512 * jt:512 * jt + 512],
                            start=(jt == 0), stop=(jt == 3))
                    ot = o_pool.tile([128, 512], F32, tag="ot", name="ot")
                    nc.vector.tensor_copy(ot[:], ps[:])
                    nc.sync.dma_start(
                        out[128 * it:128 * it + 128, 512 * et:512 * et + 512], ot[:])
                return emit

            # ordered filler queue with availability gating
            fillers = []          # list of closures
            ready = []            # parallel list of bools
            drained = [0]         # next index to drain

            def add_fill(fn, is_ready=True):
                fillers.append(fn)
                ready.append(is_ready)
                return len(fillers) - 1

            def drain(n):
                k = 0
                while k < n and drained[0] < len(fillers) and ready[drained[0]]:
                    fillers[drained[0]]()
                    drained[0] += 1
                    k += 1

            def flush_to(idx):
                while drained[0] < idx:
                    assert ready[drained[0]], f"filler {drained[0]} not ready"
                    fillers[drained[0]]()
                    drained[0] += 1

            # build the static filler order
            levels = {}
            for m in range(1, 4):
                add_fill(proj_qk(4 + m, 0))
                add_fill(proj_qk(m, 0))
                levels[(0, m)] = len(fillers)
            op_idx = {}
            for J in range(1, NT):
                for it in range(4 * J, 4 * J + 4):
                    add_fill(proj_v(it))
                for jt in (4, 5, 6, 7, 0, 1, 2, 3):
                    add_fill(proj_qk(jt, J))
                levels[(J, 0)] = len(fillers)
                # out-projection of block J-1 (gated on normalize of J-1)
                ops = []
                for it in range(4 * (J - 1), 4 * (J - 1) + 4):
                    for et in range(2):
                        ops.append(add_fill(outproj(it, et), is_ready=False))
                op_idx[J - 1] = ops
            ops = []
            for it in range(4 * 3, 4 * 3 + 4):
                for et in range(2):
                    ops.append(add_fill(outproj(it, et), is_ready=False))
            op_idx[3] = ops

            # ---------- phase 0: first projections ----------
            proj_qk(4, 0)()   # k of head-pair 0
            proj_qk(0, 0)()   # q of head-pair 0
            for it in range(4):
                proj_v(it)()

            # ---------- main attention loop ----------
            ESC = 0.125
            pending_norm = []
            for J in range(NT):
                pace = 1 if J <= 1 else (2 if J == 2 else 8)
                for m in range(4):
                    lvl = levels.get((J, m))
                    if lvl is not None:
                        flush_to(lvl)
                    psy = {0: y_psum.tile([VP, JW], F32, tag="ya", name="psya"),
                           64: y_psum.tile([VP, JW], F32, tag="yb", name="psyb")}
                    nki = 4 * J + 4
                    p8v = None
                    if J > 0:
                        p8 = p8_pool.tile([128, 2 * 6 * 2 * JW], FP8E5, tag="p8")
                        p8v = p8[:].rearrange("p (o pr t q) -> p o pr t q",
                                              o=2, pr=6, t=2, q=JW)
                    pd = pd_pool.tile([128, 2 * 4 * JW], BF16, tag="pd")
                    pdv = pd[:].rearrange("p (o r q) -> p o r q", o=2, r=4, q=JW)

                    first_pv = {0: True, 64: True}
                    n_pv = (2 * J) + 4          # DR pairs + diag singles per off
                    pv_done = {0: 0, 64: 0}
                    pending_pv = []

                    def emit_pv():
                        for fn in pending_pv:
                            fn()
                        pending_pv.clear()

                    for i in range(nki):
                        r = i - 4 * J
                        lo = 128 * r if r > 0 else 0
                        sch = s_psum.tile([128, 2 * JW], F32, tag="s", name="S")
                        schv = sch[:].rearrange("p (o q) -> p o q", o=2, q=JW)
                        for oi, off in enumerate((0, 64)):
                            nc.tensor.matmul(
                                schv[:, oi, lo:JW],
                                qk_sb[4 + m][off:off + 64, 128 * i:128 * i + 128],
                                qk_sb[m][off:off + 64, JW * J + lo:JW * J + JW],
                                start=True, stop=True)
                        if r >= 0:
                            for oi in range(2):
                                nc.vector.tensor_add(
                                    schv[:, oi, 128 * r:128 * r + 128],
                                    schv[:, oi, 128 * r:128 * r + 128],
                                    dmask_sb[:])
                            nc.scalar.activation(
                                pdv[:, :, r, lo:JW], schv[:, :, lo:JW], EXP, scale=ESC)

                            def mk_diag(i=i, r=r, lo=lo):
                                def go():
                                    for oi, off in enumerate((0, 64)):
                                        h = 2 * m + oi
                                        pv_done[off] += 1
                                        nc.tensor.matmul(
                                            psy[off][0:65, lo:JW],
                                            vbv[:, i, h, :],
                                            pdv[:, oi, r, lo:JW],
                                            start=first_pv[off],
                                            stop=(pv_done[off] == n_pv),
                                            skip_group_check=True)
                                        first_pv[off] = False
                                return go
                            pending_pv.append(mk_diag())
                        else:
                            nc.scalar.activation(
                                p8v[:, :, i // 2, i % 2, :], schv[:, :, :], EXP, scale=ESC)
                            if i % 2 == 1:
                                def mk_pair(i=i):
                                    def go():
                                        for oi, off in enumerate((0, 64)):
                                            h = 2 * m + oi
                                            pv_done[off] += 1
                                            nc.tensor.matmul(
                                                psy[off][:],
                                                v8v[:, i // 2, h, :, :],
                                                p8v[:, oi, i // 2, :, :],
                                                start=first_pv[off],
                                                stop=(pv_done[off] == n_pv),
                                                perf_mode=DR, skip_group_check=True)
                                            first_pv[off] = False
                                    return go
                                pending_pv.append(mk_pair())
                        # filler work covers the exp latency before PV lands
                        if i % pace == 0:
                            drain(1)
                        if i == 1 and pending_norm:
                            for fn in pending_norm:
                                fn()
                            pending_norm.clear()
                        if i >= 1:
                            emit_pv()
                    emit_pv()

                    def mk_norm(m=m, J=J, psy=psy):
                        def go():
                            for off in (0, 64):
                                rsr = fin_pool.tile([1, JW], F32, tag="rsr", name="rsr")
                                nc.vector.tensor_copy(rsr[:], psy[off][64:65, :])
                                rrow = fin_pool.tile([1, JW], F32, tag="rrow", name="rrow")
                                nc.vector.reciprocal_approx_fast(rrow[:], rsr[:])
                                rec = fin_pool.tile([64, JW], F32, tag="rec", name="rec")
                                nc.gpsimd.partition_broadcast(rec[:], rrow[:])
                                nc.vector.tensor_mul(
                                    y_sb[m][off:off + 64, JW * J:JW * J + JW],
                                    psy[off][0:64, :], rec[:])
                            if m == 3:
                                # y(J) now complete: release its out-proj fillers
                                for idx in op_idx[J]:
                                    ready[idx] = True
                        return go
                    pending_norm.append(mk_norm())
                # last block: no next chunk loop to host the deferred normalize
                if J == NT - 1:
                    for fn in pending_norm:
                        fn()
                    pending_norm.clear()
            # tail: remaining out-projection (+ any stragglers)
            flush_to(len(fillers))
    nc.compile()
    return nc


def _host_masks():
    a = np.arange(128, dtype=np.int64)[:, None]
    b = np.arange(128, dtype=np.int64)[None, :]
    return np.where(a <= b, np.float32(0.0), np.float32(MASK_VAL))


def _pack_ct(arr):
    """[1024, n] f32 -> [128, 8*n] bf16 with c = 128*ct + p packing."""
    n = arr.shape[1]
    return np.ascontiguousarray(
        arr.reshape(NC_, 128, n).transpose(1, 0, 2).reshape(128, NC_ * n)
        .astype(BFNP))


def _make_in_map(core, x, w_qkv, w_out):
    b, g = divmod(core, 2)
    xT = np.ascontiguousarray(x[b].T)
    wqk = np.concatenate(
        [w_qkv[:, 512 * g:512 * g + 512],
         w_qkv[:, 1024 + 512 * g:1024 + 512 * g + 512]], axis=1)
    wv = w_qkv[:, 2048 + 512 * g:2048 + 512 * g + 512]
    wout_s = np.ascontiguousarray(w_out[512 * g:512 * g + 512, :]).astype(BFNP)
    return dict(
        xtb=_pack_ct(xT),
        wqk=_pack_ct(wqk),
        wv=_pack_ct(wv),
        wout=wout_s,
        dmask=_host_masks(),
        ones_row=np.ones((1, 64), np.float32))


def kernel(x, w_qkv, w_out):
    x = np.ascontiguousarray(x, dtype=np.float32)
    w_qkv = np.ascontiguousarray(w_qkv, dtype=np.float32)
    w_out = np.ascontiguousarray(w_out, dtype=np.float32)

    if "nc" not in _cache:
        _cache["nc"] = _build()
    nc = _cache["nc"]

    in_maps = [_make_in_map(core, x, w_qkv, w_out) for core in range(8)]

    res = run_bass_kernel_spmd(nc, in_maps, core_ids=list(range(8)))
    out = np.empty((B, T, C), np.float32)
    for b in range(B):
        out[b] = res.results[2 * b]["out"] + res.results[2 * b + 1]["out"]
    return out


# revision 15
# speedup vs baseline: 1.2429x; 1.0418x over previous
"""Causal self-attention (B=4, T=2048, C=1024, H=16, Dh=64) on 8 trn2 NeuronCores.

Sharding: core = 2*b + g  (b = batch 0..3, g = head-group 0..1, 8 heads each).
Each core computes its batch's QKV projection for its 8 heads, causal
attention, and a partial out-projection; host sums the two head-group
partials per batch (the tensor-parallel "all-reduce").

v3 design (per core), single software-pipelined loop:
  - All projections bf16 (fp8 there costs ~3-5% output error).
  - q^T/k^T bf16 [j, t]; S^T[tk, tq] per head-pair computed into fp32-psum
    chunks [128, 2, 512] (both heads) -> ONE fused exp (ACT) per tk tile.
  - exp writes P: diagonal-straddling tiles -> bf16; strictly-causal tiles ->
    fp8 e5m2 (no max-subtraction; e5m2 spans e^-14..e^11; softmax averaging
    damps the 2-bit mantissa noise).
  - PV: off-diag via fp8 DoubleRow over tk-tile pairs (V e4m3 lhsT, d-dim
    padded to 80 for the 16B DoubleRow stride rule, ones col 64 = rowsum);
    diag tiles bf16. PV emission lags exp by one chunk so the PE never
    blocks on ACT.
  - Projection / V / out-projection matmul chunks are interleaved between
    attention chunks from an ordered filler queue, keeping the PE
    continuously busy (full 2.4 GHz pstate) across the whole kernel.
  - reciprocal + K=1 ones matmul broadcasts 1/rowsum; DVE mul -> y^T bf16.
  - out-projection bf16, one tq-block behind attention.
"""

import sys

for _p in ("/opt/trn_rl_repo", "/opt/pypackages"):
    if _p not in sys.path:
        sys.path.append(_p)

import numpy as np
import ml_dtypes
from contextlib import ExitStack

import concourse.bass as bass
import concourse.tile as tile
from concourse import bacc, mybir
from concourse.bass_utils import run_bass_kernel_spmd

B, T, C = 4, 2048, 1024
H, DH = 16, 64
HG = 8          # heads per core
JW = 512        # tq tile width
NT = T // JW    # 4 tq tiles
NK = T // 128   # 16 tk tiles
NC_ = C // 128  # 8 contraction tiles
VP = 80         # padded V free dim (16B-aligned for DoubleRow)
MASK_VAL = -1.0e8
F32 = mybir.dt.float32
F32R = mybir.dt.float32r
BF16 = mybir.dt.bfloat16
FP8E4 = mybir.dt.float8e4
FP8E5 = mybir.dt.float8e5
EXP = mybir.ActivationFunctionType.Exp
DR = mybir.MatmulPerfMode.DoubleRow

E4NP = ml_dtypes.float8_e4m3
E5NP = ml_dtypes.float8_e5m2
BFNP = ml_dtypes.bfloat16

_cache = {}


def _build():
    nc = bacc.Bacc("TRN2", target_bir_lowering=False, debug=False, num_devices=8)
    xtb_d = nc.dram_tensor("xtb", [128, NC_ * T], BF16, kind="ExternalInput").ap()
    wqk_d = nc.dram_tensor("wqk", [128, NC_ * 1024], BF16, kind="ExternalInput").ap()
    wv_d = nc.dram_tensor("wv", [128, NC_ * 512], BF16, kind="ExternalInput").ap()
    wout_d = nc.dram_tensor("wout", [512, C], BF16, kind="ExternalInput").ap()
    dmask_d = nc.dram_tensor("dmask", [128, 128], BF16, kind="ExternalInput").ap()
    ident_d = nc.dram_tensor("ident", [128, 128], BF16, kind="ExternalInput").ap()
    ones_row = nc.dram_tensor("ones_row", [1, 64], F32R, kind="ExternalInput").ap()
    out = nc.dram_tensor("out", [T, C], F32, kind="ExternalOutput").ap()

    with tile.TileContext(nc) as tc:
        with ExitStack() as ctx:
            ctx.enter_context(nc.allow_low_precision(reason="fp8/bf16 mixed precision intended"))
            # ---- persistent SBUF ----
            big = ctx.enter_context(tc.tile_pool(name="big", bufs=1))
            qk_sb = [big.tile([128, T], BF16, tag=f"qk{j}", name=f"qk_sb{j}") for j in range(8)]
            v8 = big.tile([128, 8 * HG * 2 * VP], FP8E4, tag="v8", name="v8")
            v8v = v8[:].rearrange("p (pr h t d) -> p pr h t d", pr=8, h=HG, t=2, d=VP)
            vb = big.tile([128, NK * HG * 65], BF16, tag="vb", name="vb")
            vbv = vb[:].rearrange("p (i h d) -> p i h d", i=NK, h=HG, d=65)
            y_sb = [big.tile([128, T], BF16, tag=f"y{m}", name=f"y_sb{m}") for m in range(4)]
            onesr = big.tile([1, 64], F32R, tag="onesr", name="onesr")
            dmask_sb = big.tile([128, 128], BF16, tag="dm", name="dmask_sb")
            ident_sb = big.tile([128, 128], BF16, tag="ident", name="ident_sb")
            xtb = big.tile([128, NC_ * T], BF16, tag="xtb", name="xtb")
            wqkb = big.tile([128, NC_ * 1024], BF16, tag="wqkb", name="wqkb")
            wvb = big.tile([128, NC_ * 512], BF16, tag="wvb", name="wvb")
            wo_sb = big.tile([128, 2 * 4 * 512], BF16, tag="wo", name="wo_sb")
            wov = wo_sb[:].rearrange("p (e j) -> p e j", e=2, j=4 * 512)

            nc.gpsimd.dma_start(onesr[:], ones_row[:])
            nc.gpsimd.dma_start(dmask_sb[:], dmask_d[:])
            nc.gpsimd.dma_start(ident_sb[:], ident_d[:])
            xtv = xtb[:].rearrange("p (c t) -> p c t", c=NC_, t=T)
            xtdv = xtb_d[:].rearrange("p (c t) -> p c t", c=NC_, t=T)
            wqkv = wqkb[:].rearrange("p (c j) -> p c j", c=NC_, j=1024)
            wqkdv = wqk_d[:].rearrange("p (c j) -> p c j", c=NC_, j=1024)
            # split DMAs, ordered so the first projection chunks start early
            for jt in (4, 0):
                nc.gpsimd.dma_start(wqkv[:, :, 128 * jt:128 * jt + 128],
                                    wqkdv[:, :, 128 * jt:128 * jt + 128])
            nc.gpsimd.dma_start(xtv[:, :, 0:JW], xtdv[:, :, 0:JW])
            nc.gpsimd.dma_start(wvb[:], wv_d[:])
            for jt in (5, 1, 6, 2, 7, 3):
                nc.gpsimd.dma_start(wqkv[:, :, 128 * jt:128 * jt + 128],
                                    wqkdv[:, :, 128 * jt:128 * jt + 128])
            for tt in range(1, NT):
                nc.gpsimd.dma_start(xtv[:, :, JW * tt:JW * tt + JW],
                                    xtdv[:, :, JW * tt:JW * tt + JW])
            for jt in range(4):
                for et in range(2):
                    nc.gpsimd.dma_start(
                        wov[:, et, 512 * jt:512 * jt + 512],
                        wout_d[128 * jt:128 * jt + 128, 512 * et:512 * et + 512])
            wvv = wvb[:].rearrange("p (c j) -> p c j", c=NC_, j=512)
            for pr in range(8):
                nc.vector.memset(v8v[:, pr, :, :, 64], 1.0)
                nc.vector.memset(v8v[:, pr, :, :, 65:VP], 0.0)
            nc.vector.memset(vbv[:, :, :, 64], 1.0)

            # P buffers
            p8_pool = ctx.enter_context(tc.tile_pool(name="p8", bufs=2))
            pd_pool = ctx.enter_context(tc.tile_pool(name="pd", bufs=2))
            fin_pool = ctx.enter_context(tc.tile_pool(name="fin", bufs=2))
            # PSUM: s 2x2 banks + ya/yb 1 each + aux 2 = 8 banks
            s_psum = ctx.enter_context(tc.tile_pool(name="s_psum", bufs=2, space="PSUM"))
            y_psum = ctx.enter_context(tc.tile_pool(name="y_psum", bufs=1, space="PSUM"))
            aux_psum = ctx.enter_context(tc.tile_pool(name="aux_psum", bufs=2, space="PSUM"))
            o_pool = ctx.enter_context(tc.tile_pool(name="o", bufs=2))

            # ---------- PE work-unit emitters ----------
            def proj_qk(jt, tt):
                def emit():
                    ps = aux_psum.tile([128, JW], F32, tag="aux", name="psaux")
                    for ct in range(NC_):
                        nc.tensor.matmul(
                            ps[:], wqkv[:, ct, 128 * jt:128 * jt + 128],
                            xtv[:, ct, JW * tt:JW * tt + JW],
                            start=(ct == 0), stop=(ct == NC_ - 1))
                    nc.scalar.copy(qk_sb[jt][:, JW * tt:JW * tt + JW], ps[:])
                return emit

            def proj_v(it):
                def emit():
                    ps = aux_psum.tile([128, JW], F32, tag="aux", name="psaux")
                    for ct in range(NC_):
                        nc.tensor.matmul(
                            ps[:], xtv[:, ct, 128 * it:128 * it + 128],
                            wvv[:, ct, :],
                            start=(ct == 0), stop=(ct == NC_ - 1))
                    psv = ps[:].rearrange("p (h d) -> p h d", h=HG, d=64)
                    nc.scalar.copy(v8v[:, it // 2, :, it % 2, 0:64], psv)
                    nc.vector.tensor_copy(vbv[:, it, :, 0:64], psv)
                return emit

            def outproj(it, et):
                def emit():
                    ps = aux_psum.tile([128, JW], F32, tag="aux", name="psaux")
                    for jt in range(4):
                        nc.tensor.matmul(
                            ps[:], y_sb[jt][:, 128 * it:128 * it + 128],
                            wov[:, et, 512 * jt:512 * jt + 512],
                            start=(jt == 0), stop=(jt == 3))
                    ot = o_pool.tile([128, 512], F32, tag="ot", name="ot")
                    nc.vector.tensor_copy(ot[:], ps[:])
                    nc.sync.dma_start(
                        out[128 * it:128 * it + 128, 512 * et:512 * et + 512], ot[:])
                return emit

            # ordered filler queue with availability gating
            fillers = []          # list of closures
            ready = []            # parallel list of bools
            drained = [0]         # next index to drain

            def add_fill(fn, is_ready=True):
                fillers.append(fn)
                ready.append(is_ready)
                return len(fillers) - 1

            def drain(n):
                k = 0
                while k < n and drained[0] < len(fillers) and ready[drained[0]]:
                    fillers[drained[0]]()
                    drained[0] += 1
                    k += 1

            def flush_to(idx):
                while drained[0] < idx:
                    assert ready[drained[0]], f"filler {drained[0]} not ready"
                    fillers[drained[0]]()
                    drained[0] += 1

            # build the static filler order
            levels = {}
            for m in range(1, 4):
                add_fill(proj_qk(4 + m, 0))
                add_fill(proj_qk(m, 0))
                levels[(0, m)] = len(fillers)
            op_idx = {}
            for J in range(1, NT):
                for it in range(4 * J, 4 * J + 4):
                    add_fill(proj_v(it))
                for jt in (4, 5, 6, 7, 0, 1, 2, 3):
                    add_fill(proj_qk(jt, J))
                levels[(J, 0)] = len(fillers)
                # out-projection of block J-1 (gated on normalize of J-1)
                ops = []
                for it in range(4 * (J - 1), 4 * (J - 1) + 4):
                    for et in range(2):
                        ops.append(add_fill(outproj(it, et), is_ready=False))
                op_idx[J - 1] = ops
            ops = []
            for it in range(4 * 3, 4 * 3 + 4):
                for et in range(2):
                    ops.append(add_fill(outproj(it, et), is_ready=False))
            op_idx[3] = ops

            # ---------- phase 0: first projections ----------
            proj_qk(4, 0)()   # k of head-pair 0
            proj_qk(0, 0)()   # q of head-pair 0
            for it in range(4):
                proj_v(it)()

            # ---------- main attention loop ----------
            ESC = 0.125
            pending_norm = []
            for J in range(NT):
                pace = 1 if J <= 1 else (2 if J == 2 else 8)
                for m in range(4):
                    lvl = levels.get((J, m))
                    if lvl is not None:
                        flush_to(lvl)
                    psy = {0: y_psum.tile([VP, JW], F32, tag="ya", name="psya"),
                           64: y_psum.tile([VP, JW], F32, tag="yb", name="psyb")}
                    nki = 4 * J + 4
                    p8v = None
                    if J > 0:
                        p8 = p8_pool.tile([128, 2 * 6 * 2 * JW], FP8E5, tag="p8")
                        p8v = p8[:].rearrange("p (o pr t q) -> p o pr t q",
                                              o=2, pr=6, t=2, q=JW)
                    pd = pd_pool.tile([128, 2 * 4 * JW], BF16, tag="pd")
                    pdv = pd[:].rearrange("p (o r q) -> p o r q", o=2, r=4, q=JW)

                    first_pv = {0: True, 64: True}
                    n_pv = (2 * J) + 4          # DR pairs + diag singles per off
                    pv_done = {0: 0, 64: 0}
                    pending_pv = []

                    def emit_pv():
                        for fn in pending_pv:
                            fn()
                        pending_pv.clear()

                    for i in range(nki):
                        r = i - 4 * J
                        lo = 128 * r if r > 0 else 0
                        sch = s_psum.tile([128, 2 * JW], F32, tag="s", name="S")
                        schv = sch[:].rearrange("p (o q) -> p o q", o=2, q=JW)
                        diag = r >= 0
                        for oi, off in enumerate((0, 64)):
                            nc.tensor.matmul(
                                schv[:, oi, lo:JW],
                                qk_sb[4 + m][off:off + 64, 128 * i:128 * i + 128],
                                qk_sb[m][off:off + 64, JW * J + lo:JW * J + JW],
                                start=True, stop=not diag,
                                skip_group_check=diag)
                        if diag:
                            for oi in range(2):
                                nc.tensor.matmul(
                                    schv[:, oi, 128 * r:128 * r + 128],
                                    ident_sb[:], dmask_sb[:],
                                    start=False, stop=True,
                                    skip_group_check=True)
                            nc.scalar.activation(
                                pdv[:, :, r, lo:JW], schv[:, :, lo:JW], EXP, scale=ESC)

                            def mk_diag(i=i, r=r, lo=lo):
                                def go():
                                    for oi, off in enumerate((0, 64)):
                                        h = 2 * m + oi
                                        pv_done[off] += 1
                                        nc.tensor.matmul(
                                            psy[off][0:65, lo:JW],
                                            vbv[:, i, h, :],
                                            pdv[:, oi, r, lo:JW],
                                            start=first_pv[off],
                                            stop=(pv_done[off] == n_pv),
                                            skip_group_check=True)
                                        first_pv[off] = False
                                return go
                            pending_pv.append(mk_diag())
                        else:
                            nc.scalar.activation(
                                p8v[:, :, i // 2, i % 2, :], schv[:, :, :], EXP, scale=ESC)
                            if i % 2 == 1:
                                def mk_pair(i=i):
                                    def go():
                                        for oi, off in enumerate((0, 64)):
                                            h = 2 * m + oi
                                            pv_done[off] += 1
                                            nc.tensor.matmul(
                                                psy[off][:],
                                                v8v[:, i // 2, h, :, :],
                                                p8v[:, oi, i // 2, :, :],
                                                start=first_pv[off],
                                                stop=(pv_done[off] == n_pv),
                                                perf_mode=DR, skip_group_check=True)
                                            first_pv[off] = False
                                    return go
                                pending_pv.append(mk_pair())
                        # filler work covers the exp latency before PV lands
                        if i % pace == 0:
                            drain(1)
                        if i == 1 and pending_norm:
                            for fn in pending_norm:
                                fn()
                            pending_norm.clear()
                        if i >= 1:
                            emit_pv()
                    emit_pv()

                    def mk_norm(m=m, J=J, psy=psy):
                        def go():
                            for off in (0, 64):
                                rsr = fin_pool.tile([1, JW], F32, tag="rsr", name="rsr")
                                nc.vector.tensor_copy(rsr[:], psy[off][64:65, :])
                                rrow = fin_pool.tile([1, JW], F32, tag="rrow", name="rrow")
                                nc.vector.reciprocal_approx_fast(rrow[:], rsr[:])
                                rec = fin_pool.tile([64, JW], F32, tag="rec", name="rec")
                                nc.gpsimd.partition_broadcast(rec[:], rrow[:])
                                nc.vector.tensor_mul(
                                    y_sb[m][off:off + 64, JW * J:JW * J + JW],
                                    psy[off][0:64, :], rec[:])
                            if m == 3:
                                # y(J) now complete: release its out-proj fillers
                                for idx in op_idx[J]:
                                    ready[idx] = True
                        return go
                    pending_norm.append(mk_norm())
                # last block: no next chunk loop to host the deferred normalize
                if J == NT - 1:
                    for fn in pending_norm:
                        fn()
                    pending_norm.clear()
            # tail: remaining out-projection (+ any stragglers)
            flush_to(len(fillers))
    nc.compile()
    return nc


def _host_masks():
    a = np.arange(128, dtype=np.int64)[:, None]
    b = np.arange(128, dtype=np.int64)[None, :]
    return np.where(a <= b, np.float32(0.0), np.float32(MASK_VAL)).astype(BFNP)


def _pack_ct(arr):
    """[1024, n] f32 -> [128, 8*n] bf16 with c = 128*ct + p packing."""
    n = arr.shape[1]
    return np.ascontiguousarray(
        arr.reshape(NC_, 128, n).transpose(1, 0, 2).reshape(128, NC_ * n)
        .astype(BFNP))


def _make_in_map(core, x, w_qkv, w_out):
    b, g = divmod(core, 2)
    xT = np.ascontiguousarray(x[b].T)
    wqk = np.concatenate(
        [w_qkv[:, 512 * g:512 * g + 512],
         w_qkv[:, 1024 + 512 * g:1024 + 512 * g + 512]], axis=1)
    wv = w_qkv[:, 2048 + 512 * g:2048 + 512 * g + 512]
    wout_s = np.ascontiguousarray(w_out[512 * g:512 * g + 512, :]).astype(BFNP)
    return dict(
        xtb=_pack_ct(xT),
        wqk=_pack_ct(wqk),
        wv=_pack_ct(wv),
        wout=wout_s,
        dmask=_host_masks(),
        ident=np.eye(128, dtype=np.float32).astype(BFNP),
        ones_row=np.ones((1, 64), np.float32))


def kernel(x, w_qkv, w_out):
    x = np.ascontiguousarray(x, dtype=np.float32)
    w_qkv = np.ascontiguousarray(w_qkv, dtype=np.float32)
    w_out = np.ascontiguousarray(w_out, dtype=np.float32)

    if "nc" not in _cache:
        _cache["nc"] = _build()
    nc = _cache["nc"]

    in_maps = [_make_in_map(core, x, w_qkv, w_out) for core in range(8)]

    res = run_bass_kernel_spmd(nc, in_maps, core_ids=list(range(8)))
    out = np.empty((B, T, C), np.float32)
    for b in range(B):
        out[b] = res.results[2 * b]["out"] + res.results[2 * b + 1]["out"]
    return out


# revision 18
# speedup vs baseline: 1.2717x; 1.0232x over previous
"""Causal self-attention (B=4, T=2048, C=1024, H=16, Dh=64) on 8 trn2 NeuronCores.

Sharding: core = 2*b + g  (b = batch 0..3, g = head-group 0..1, 8 heads each).
Each core computes its batch's QKV projection for its 8 heads, causal
attention, and a partial out-projection; host sums the two head-group
partials per batch (the tensor-parallel "all-reduce").

v3 design (per core), single software-pipelined loop:
  - All projections bf16 (fp8 there costs ~3-5% output error).
  - q^T/k^T bf16 [j, t]; S^T[tk, tq] per head-pair computed into fp32-psum
    chunks [128, 2, 512] (both heads) -> ONE fused exp (ACT) per tk tile.
  - exp writes P: diagonal-straddling tiles -> bf16; strictly-causal tiles ->
    fp8 e5m2 (no max-subtraction; e5m2 spans e^-14..e^11; softmax averaging
    damps the 2-bit mantissa noise).
  - PV: off-diag via fp8 DoubleRow over tk-tile pairs (V e4m3 lhsT, d-dim
    padded to 80 for the 16B DoubleRow stride rule, ones col 64 = rowsum);
    diag tiles bf16. PV emission lags exp by one chunk so the PE never
    blocks on ACT.
  - Projection / V / out-projection matmul chunks are interleaved between
    attention chunks from an ordered filler queue, keeping the PE
    continuously busy (full 2.4 GHz pstate) across the whole kernel.
  - reciprocal + K=1 ones matmul broadcasts 1/rowsum; DVE mul -> y^T bf16.
  - out-projection bf16, one tq-block behind attention.
"""

import sys

for _p in ("/opt/trn_rl_repo", "/opt/pypackages"):
    if _p not in sys.path:
        sys.path.append(_p)

import numpy as np
import ml_dtypes
from contextlib import ExitStack

import concourse.bass as bass
import concourse.tile as tile
from concourse import bacc, mybir
from concourse.bass_utils import run_bass_kernel_spmd

B, T, C = 4, 2048, 1024
H, DH = 16, 64
HG = 8          # heads per core
JW = 512        # tq tile width
NT = T // JW    # 4 tq tiles
NK = T // 128   # 16 tk tiles
NC_ = C // 128  # 8 contraction tiles
VP = 80         # padded V free dim (16B-aligned for DoubleRow)
MASK_VAL = -1.0e8
F32 = mybir.dt.float32
F32R = mybir.dt.float32r
BF16 = mybir.dt.bfloat16
FP8E4 = mybir.dt.float8e4
FP8E5 = mybir.dt.float8e5
EXP = mybir.ActivationFunctionType.Exp
DR = mybir.MatmulPerfMode.DoubleRow

E4NP = ml_dtypes.float8_e4m3
E5NP = ml_dtypes.float8_e5m2
BFNP = ml_dtypes.bfloat16

_cache = {}


def _build():
    nc = bacc.Bacc("TRN2", target_bir_lowering=False, debug=False, num_devices=8)
    xtb_d = nc.dram_tensor("xtb", [128, NC_ * T], BF16, kind="ExternalInput").ap()
    wqk_d = nc.dram_tensor("wqk", [128, NC_ * 1024], BF16, kind="ExternalInput").ap()
    wv_d = nc.dram_tensor("wv", [128, NC_ * 512], BF16, kind="ExternalInput").ap()
    wout_d = nc.dram_tensor("wout", [512, C], BF16, kind="ExternalInput").ap()
    dmask_d = nc.dram_tensor("dmask", [128, 128], BF16, kind="ExternalInput").ap()
    ident_d = nc.dram_tensor("ident", [128, 128], BF16, kind="ExternalInput").ap()
    ones_row = nc.dram_tensor("ones_row", [1, 64], F32R, kind="ExternalInput").ap()
    out = nc.dram_tensor("out", [T, C], F32, kind="ExternalOutput").ap()

    with tile.TileContext(nc) as tc:
        with ExitStack() as ctx:
            ctx.enter_context(nc.allow_low_precision(reason="fp8/bf16 mixed precision intended"))
            # ---- persistent SBUF ----
            big = ctx.enter_context(tc.tile_pool(name="big", bufs=1))
            qk_sb = [big.tile([128, T], BF16, tag=f"qk{j}", name=f"qk_sb{j}") for j in range(8)]
            v8 = big.tile([128, 8 * HG * 2 * VP], FP8E4, tag="v8", name="v8")
            v8v = v8[:].rearrange("p (pr h t d) -> p pr h t d", pr=8, h=HG, t=2, d=VP)
            vb = big.tile([128, NK * HG * 65], BF16, tag="vb", name="vb")
            vbv = vb[:].rearrange("p (i h d) -> p i h d", i=NK, h=HG, d=65)
            y_sb = [big.tile([128, T], BF16, tag=f"y{m}", name=f"y_sb{m}") for m in range(4)]
            onesr = big.tile([1, 64], F32R, tag="onesr", name="onesr")
            dmask_sb = big.tile([128, 128], BF16, tag="dm", name="dmask_sb")
            ident_sb = big.tile([128, 128], BF16, tag="ident", name="ident_sb")
            xtb = big.tile([128, NC_ * T], BF16, tag="xtb", name="xtb")
            wqkb = big.tile([128, NC_ * 1024], BF16, tag="wqkb", name="wqkb")
            wvb = big.tile([128, NC_ * 512], BF16, tag="wvb", name="wvb")
            wo_sb = big.tile([128, 2 * 4 * 512], BF16, tag="wo", name="wo_sb")
            wov = wo_sb[:].rearrange("p (e j) -> p e j", e=2, j=4 * 512)

            nc.gpsimd.dma_start(onesr[:], ones_row[:])
            nc.gpsimd.dma_start(dmask_sb[:], dmask_d[:])
            nc.gpsimd.dma_start(ident_sb[:], ident_d[:])
            # packed layouts: xt = [p, (tt ct t')], wqk = [p, (jt ct j')]
            xtv = xtb[:].rearrange("p (s c t) -> p s c t", s=NT, c=NC_, t=JW)
            xtdv = xtb_d[:].rearrange("p (s c t) -> p s c t", s=NT, c=NC_, t=JW)
            wqkv = wqkb[:].rearrange("p (g c j) -> p g c j", g=8, c=NC_, j=128)
            wqkdv = wqk_d[:].rearrange("p (g c j) -> p g c j", g=8, c=NC_, j=128)
            # split DMAs, ordered so the first projection chunks start early
            for jt in (4, 0):
                nc.gpsimd.dma_start(wqkv[:, jt], wqkdv[:, jt])
            nc.gpsimd.dma_start(xtv[:, 0], xtdv[:, 0])
            nc.gpsimd.dma_start(wvb[:], wv_d[:])
            for jt in (5, 1, 6, 2, 7, 3):
                nc.gpsimd.dma_start(wqkv[:, jt], wqkdv[:, jt])
            for tt in range(1, NT):
                nc.gpsimd.dma_start(xtv[:, tt], xtdv[:, tt])
            for jt in range(4):
                for et in range(2):
                    nc.gpsimd.dma_start(
                        wov[:, et, 512 * jt:512 * jt + 512],
                        wout_d[128 * jt:128 * jt + 128, 512 * et:512 * et + 512])
            wvv = wvb[:].rearrange("p (c j) -> p c j", c=NC_, j=512)
            for pr in range(8):
                nc.vector.memset(v8v[:, pr, :, :, 64], 1.0)
                nc.vector.memset(v8v[:, pr, :, :, 65:VP], 0.0)
            nc.vector.memset(vbv[:, :, :, 64], 1.0)

            # P buffers
            p8_pool = ctx.enter_context(tc.tile_pool(name="p8", bufs=2))
            pd_pool = ctx.enter_context(tc.tile_pool(name="pd", bufs=2))
            fin_pool = ctx.enter_context(tc.tile_pool(name="fin", bufs=2))
            # PSUM: s 2x2 banks + ya/yb 1 each + aux 2 = 8 banks
            s_psum = ctx.enter_context(tc.tile_pool(name="s_psum", bufs=2, space="PSUM"))
            y_psum = ctx.enter_context(tc.tile_pool(name="y_psum", bufs=1, space="PSUM"))
            aux_psum = ctx.enter_context(tc.tile_pool(name="aux_psum", bufs=2, space="PSUM"))
            o_pool = ctx.enter_context(tc.tile_pool(name="o", bufs=2))

            # ---------- PE work-unit emitters ----------
            def proj_qk(jt, tt):
                def emit():
                    ps = aux_psum.tile([128, JW], F32, tag="aux", name="psaux")
                    for ct in range(NC_):
                        nc.tensor.matmul(
                            ps[:], wqkv[:, jt, ct, :],
                            xtv[:, tt, ct, :],
                            start=(ct == 0), stop=(ct == NC_ - 1))
                    nc.scalar.copy(qk_sb[jt][:, JW * tt:JW * tt + JW], ps[:])
                return emit

            def proj_v(it):
                def emit():
                    ps = aux_psum.tile([128, JW], F32, tag="aux", name="psaux")
                    for ct in range(NC_):
                        nc.tensor.matmul(
                            ps[:], xtv[:, it // 4, ct, 128 * (it % 4):128 * (it % 4) + 128],
                            wvv[:, ct, :],
                            start=(ct == 0), stop=(ct == NC_ - 1))
                    psv = ps[:].rearrange("p (h d) -> p h d", h=HG, d=64)
                    nc.scalar.copy(v8v[:, it // 2, :, it % 2, 0:64], psv)
                    nc.vector.tensor_copy(vbv[:, it, :, 0:64], psv)
                return emit

            def outproj(it, et):
                def emit():
                    ps = aux_psum.tile([128, JW], F32, tag="aux", name="psaux")
                    for jt in range(4):
                        nc.tensor.matmul(
                            ps[:], y_sb[jt][:, 128 * it:128 * it + 128],
                            wov[:, et, 512 * jt:512 * jt + 512],
                            start=(jt == 0), stop=(jt == 3))
                    ot = o_pool.tile([128, 512], F32, tag="ot", name="ot")
                    nc.vector.tensor_copy(ot[:], ps[:])
                    nc.sync.dma_start(
                        out[128 * it:128 * it + 128, 512 * et:512 * et + 512], ot[:])
                return emit

            # ordered filler queue with availability gating
            fillers = []          # list of closures
            ready = []            # parallel list of bools
            drained = [0]         # next index to drain

            def add_fill(fn, is_ready=True):
                fillers.append(fn)
                ready.append(is_ready)
                return len(fillers) - 1

            def drain(n):
                k = 0
                while k < n and drained[0] < len(fillers) and ready[drained[0]]:
                    fillers[drained[0]]()
                    drained[0] += 1
                    k += 1

            def flush_to(idx):
                while drained[0] < idx:
                    assert ready[drained[0]], f"filler {drained[0]} not ready"
                    fillers[drained[0]]()
                    drained[0] += 1

            # build the static filler order
            levels = {}
            for m in range(1, 4):
                add_fill(proj_qk(4 + m, 0))
                add_fill(proj_qk(m, 0))
                levels[(0, m)] = len(fillers)
            op_idx = {}
            for J in range(1, NT):
                for it in range(4 * J, 4 * J + 4):
                    add_fill(proj_v(it))
                for jt in (4, 5, 6, 7, 0, 1, 2, 3):
                    add_fill(proj_qk(jt, J))
                levels[(J, 0)] = len(fillers)
                # out-projection of block J-1 (gated on normalize of J-1)
                ops = []
                for it in range(4 * (J - 1), 4 * (J - 1) + 4):
                    for et in range(2):
                        ops.append(add_fill(outproj(it, et), is_ready=False))
                op_idx[J - 1] = ops
            ops = []
            for it in range(4 * 3, 4 * 3 + 4):
                for et in range(2):
                    ops.append(add_fill(outproj(it, et), is_ready=False))
            op_idx[3] = ops

            # ---------- phase 0: first projections ----------
            proj_qk(4, 0)()   # k of head-pair 0
            proj_qk(0, 0)()   # q of head-pair 0
            for it in range(4):
                proj_v(it)()

            # ---------- main attention loop ----------
            ESC = 0.125
            pending_norm = []

            def emit_s(J, m, i):
                r = i - 4 * J
                lo = 128 * r if r > 0 else 0
                sch = s_psum.tile([128, 2 * JW], F32, tag="s", name="S")
                schv = sch[:].rearrange("p (o q) -> p o q", o=2, q=JW)
                diag = r >= 0
                for oi, off in enumerate((0, 64)):
                    nc.tensor.matmul(
                        schv[:, oi, lo:JW],
                        qk_sb[4 + m][off:off + 64, 128 * i:128 * i + 128],
                        qk_sb[m][off:off + 64, JW * J + lo:JW * J + JW],
                        start=True, stop=not diag,
                        skip_group_check=diag)
                if diag:
                    for oi in range(2):
                        nc.tensor.matmul(
                            schv[:, oi, 128 * r:128 * r + 128],
                            ident_sb[:], dmask_sb[:],
                            start=False, stop=True,
                            skip_group_check=True)
                return schv

            blocks = [(J, m) for J in range(NT) for m in range(4)]
            prefetch = {}
            for bi, (J, m) in enumerate(blocks):
                pace = 1 if J <= 1 else (2 if J == 2 else 8)
                lvl = levels.get((J, m))
                if lvl is not None:
                    flush_to(lvl)
                psy = {0: y_psum.tile([VP, JW], F32, tag="ya", name="psya"),
                       64: y_psum.tile([VP, JW], F32, tag="yb", name="psyb")}
                nki = 4 * J + 4
                p8v = None
                if J > 0:
                    p8 = p8_pool.tile([128, 2 * 6 * 2 * JW], FP8E5, tag="p8")
                    p8v = p8[:].rearrange("p (o pr t q) -> p o pr t q",
                                          o=2, pr=6, t=2, q=JW)
                pd = pd_pool.tile([128, 2 * 4 * JW], BF16, tag="pd")
                pdv = pd[:].rearrange("p (o r q) -> p o r q", o=2, r=4, q=JW)

                first_pv = {0: True, 64: True}
                n_pv = (2 * J) + 4          # DR pairs + diag singles per off
                pv_done = {0: 0, 64: 0}
                pending_pv = []

                def emit_pv():
                    for fn in pending_pv:
                        fn()
                    pending_pv.clear()

                for i in range(nki):
                    r = i - 4 * J
                    lo = 128 * r if r > 0 else 0
                    diag = r >= 0
                    if i == 0 and (J, m) in prefetch:
                        schv = prefetch.pop((J, m))
                    else:
                        schv = emit_s(J, m, i)
                    if diag:
                        nc.scalar.activation(
                            pdv[:, :, r, lo:JW], schv[:, :, lo:JW], EXP, scale=ESC)

                        def mk_diag(i=i, r=r, lo=lo, m=m, psy=psy, pdv=pdv,
                                    first_pv=first_pv, pv_done=pv_done, n_pv=n_pv):
                            def go():
                                for oi, off in enumerate((0, 64)):
                                    h = 2 * m + oi
                                    pv_done[off] += 1
                                    nc.tensor.matmul(
                                        psy[off][0:65, lo:JW],
                                        vbv[:, i, h, :],
                                        pdv[:, oi, r, lo:JW],
                                        start=first_pv[off],
                                        stop=(pv_done[off] == n_pv),
                                        skip_group_check=True)
                                    first_pv[off] = False
                            return go
                        pending_pv.append(mk_diag())
                    else:
                        nc.scalar.activation(
                            p8v[:, :, i // 2, i % 2, :], schv[:, :, :], EXP, scale=ESC)
                        if i % 2 == 1:
                            def mk_pair(i=i, m=m, psy=psy, p8v=p8v,
                                        first_pv=first_pv, pv_done=pv_done, n_pv=n_pv):
                                def go():
                                    for oi, off in enumerate((0, 64)):
                                        h = 2 * m + oi
                                        pv_done[off] += 1
                                        nc.tensor.matmul(
                                            psy[off][:],
                                            v8v[:, i // 2, h, :, :],
                                            p8v[:, oi, i // 2, :, :],
                                            start=first_pv[off],
                                            stop=(pv_done[off] == n_pv),
                                            perf_mode=DR, skip_group_check=True)
                                        first_pv[off] = False
                                return go
                            pending_pv.append(mk_pair())
                    if i == nki - 1 and bi + 1 < len(blocks):
                        # prefetch next block's first S chunk so exp never
                        # stalls across the block boundary
                        Jn, mn = blocks[bi + 1]
                        lvln = levels.get((Jn, mn))
                        if lvln is not None:
                            flush_to(lvln)
                        prefetch[(Jn, mn)] = emit_s(Jn, mn, 0)
                    # filler work covers the exp latency before PV lands
                    if i % pace == 0:
                        drain(1)
                    if i == 1 and pending_norm:
                        for fn in pending_norm:
                            fn()
                        pending_norm.clear()
                    if i >= 1:
                        emit_pv()
                emit_pv()

                def mk_norm(m=m, J=J, psy=psy):
                    def go():
                        for off in (0, 64):
                            rsr = fin_pool.tile([1, JW], F32, tag="rsr", name="rsr")
                            nc.vector.tensor_copy(rsr[:], psy[off][64:65, :])
                            rrow = fin_pool.tile([1, JW], F32, tag="rrow", name="rrow")
                            nc.vector.reciprocal_approx_fast(rrow[:], rsr[:])
                            rec = fin_pool.tile([64, JW], F32, tag="rec", name="rec")
                            nc.gpsimd.partition_broadcast(rec[:], rrow[:])
                            nc.vector.tensor_mul(
                                y_sb[m][off:off + 64, JW * J:JW * J + JW],
                                psy[off][0:64, :], rec[:])
                        if m == 3:
                            # y(J) now complete: release its out-proj fillers
                            for idx in op_idx[J]:
                                ready[idx] = True
                    return go
                pending_norm.append(mk_norm())
                # last block: no next chunk loop to host the deferred normalize
                if (J, m) == blocks[-1]:
                    for fn in pending_norm:
                        fn()
                    pending_norm.clear()
            # tail: remaining out-projection (+ any stragglers)
            flush_to(len(fillers))
    nc.compile()
    return nc


def _host_masks():
    a = np.arange(128, dtype=np.int64)[:, None]
    b = np.arange(128, dtype=np.int64)[None, :]
    return np.where(a <= b, np.float32(0.0), np.float32(MASK_VAL)).astype(BFNP)


def _pack_ct(arr):
    """[1024, n] f32 -> [128, 8*n] bf16 with c = 128*ct + p packing."""
    n = arr.shape[1]
    return np.ascontiguousarray(
        arr.reshape(NC_, 128, n).transpose(1, 0, 2).reshape(128, NC_ * n)
        .astype(BFNP))


def _pack_blocked(arr, blk):
    """[1024, n] f32 -> [128, (n//blk)*8*blk] bf16, free = (slice, ct, blk)."""
    n = arr.shape[1]
    return np.ascontiguousarray(
        arr.reshape(NC_, 128, n // blk, blk).transpose(1, 2, 0, 3)
        .reshape(128, -1).astype(BFNP))


def _make_in_map(core, x, w_qkv, w_out):
    b, g = divmod(core, 2)
    xT = np.ascontiguousarray(x[b].T)
    wqk = np.concatenate(
        [w_qkv[:, 512 * g:512 * g + 512],
         w_qkv[:, 1024 + 512 * g:1024 + 512 * g + 512]], axis=1)
    wv = w_qkv[:, 2048 + 512 * g:2048 + 512 * g + 512]
    wout_s = np.ascontiguousarray(w_out[512 * g:512 * g + 512, :]).astype(BFNP)
    return dict(
        xtb=_pack_blocked(xT, JW),
        wqk=_pack_blocked(wqk, 128),
        wv=_pack_ct(wv),
        wout=wout_s,
        dmask=_host_masks(),
        ident=np.eye(128, dtype=np.float32).astype(BFNP),
        ones_row=np.ones((1, 64), np.float32))


def kernel(x, w_qkv, w_out):
    x = np.ascontiguousarray(x, dtype=np.float32)
    w_qkv = np.ascontiguousarray(w_qkv, dtype=np.float32)
    w_out = np.ascontiguousarray(w_out, dtype=np.float32)

    if "nc" not in _cache:
        _cache["nc"] = _build()
    nc = _cache["nc"]

    in_maps = [_make_in_map(core, x, w_qkv, w_out) for core in range(8)]

    res = run_bass_kernel_spmd(nc, in_maps, core_ids=list(range(8)))
    out = np.empty((B, T, C), np.float32)
    for b in range(B):
        out[b] = res.results[2 * b]["out"] + res.results[2 * b + 1]["out"]
    return out


# revision 20
# speedup vs baseline: 1.3067x; 1.0275x over previous
"""Causal self-attention (B=4, T=2048, C=1024, H=16, Dh=64) on 8 trn2 NeuronCores.

Sharding: core = 2*b + g  (b = batch 0..3, g = head-group 0..1, 8 heads each).
Each core computes its batch's QKV projection for its 8 heads, causal
attention, and a partial out-projection; host sums the two head-group
partials per batch (the tensor-parallel "all-reduce").

v3 design (per core), single software-pipelined loop:
  - All projections bf16 (fp8 there costs ~3-5% output error).
  - q^T/k^T bf16 [j, t]; S^T[tk, tq] per head-pair computed into fp32-psum
    chunks [128, 2, 512] (both heads) -> ONE fused exp (ACT) per tk tile.
  - exp writes P: diagonal-straddling tiles -> bf16; strictly-causal tiles ->
    fp8 e5m2 (no max-subtraction; e5m2 spans e^-14..e^11; softmax averaging
    damps the 2-bit mantissa noise).
  - PV: off-diag via fp8 DoubleRow over tk-tile pairs (V e4m3 lhsT, d-dim
    padded to 80 for the 16B DoubleRow stride rule, ones col 64 = rowsum);
    diag tiles bf16. PV emission lags exp by one chunk so the PE never
    blocks on ACT.
  - Projection / V / out-projection matmul chunks are interleaved between
    attention chunks from an ordered filler queue, keeping the PE
    continuously busy (full 2.4 GHz pstate) across the whole kernel.
  - reciprocal + K=1 ones matmul broadcasts 1/rowsum; DVE mul -> y^T bf16.
  - out-projection bf16, one tq-block behind attention.
"""

import sys

for _p in ("/opt/trn_rl_repo", "/opt/pypackages"):
    if _p not in sys.path:
        sys.path.append(_p)

import numpy as np
import ml_dtypes
from contextlib import ExitStack

import concourse.bass as bass
import concourse.tile as tile
from concourse import bacc, mybir
from concourse.bass_utils import run_bass_kernel_spmd

B, T, C = 4, 2048, 1024
H, DH = 16, 64
HG = 8          # heads per core
JW = 512        # tq tile width
NT = T // JW    # 4 tq tiles
NK = T // 128   # 16 tk tiles
NC_ = C // 128  # 8 contraction tiles
VP = 80         # padded V free dim (16B-aligned for DoubleRow)
MASK_VAL = -1.0e8
F32 = mybir.dt.float32
F32R = mybir.dt.float32r
BF16 = mybir.dt.bfloat16
FP8E4 = mybir.dt.float8e4
FP8E5 = mybir.dt.float8e5
EXP = mybir.ActivationFunctionType.Exp
DR = mybir.MatmulPerfMode.DoubleRow

E4NP = ml_dtypes.float8_e4m3
E5NP = ml_dtypes.float8_e5m2
BFNP = ml_dtypes.bfloat16

_cache = {}


def _build():
    nc = bacc.Bacc("TRN2", target_bir_lowering=False, debug=False, num_devices=8)
    xtb_d = nc.dram_tensor("xtb", [128, NC_ * T], BF16, kind="ExternalInput").ap()
    wqk_d = nc.dram_tensor("wqk", [128, NC_ * 1024], BF16, kind="ExternalInput").ap()
    wv_d = nc.dram_tensor("wv", [128, NC_ * 512], BF16, kind="ExternalInput").ap()
    wout_d = nc.dram_tensor("wout", [512, C], BF16, kind="ExternalInput").ap()
    dmask_d = nc.dram_tensor("dmask", [128, 128], BF16, kind="ExternalInput").ap()
    ident_d = nc.dram_tensor("ident", [128, 128], BF16, kind="ExternalInput").ap()
    ones_row = nc.dram_tensor("ones_row", [1, 64], F32R, kind="ExternalInput").ap()
    out = nc.dram_tensor("out", [T, C], F32, kind="ExternalOutput").ap()

    with tile.TileContext(nc) as tc:
        with ExitStack() as ctx:
            ctx.enter_context(nc.allow_low_precision(reason="fp8/bf16 mixed precision intended"))
            # ---- persistent SBUF ----
            big = ctx.enter_context(tc.tile_pool(name="big", bufs=1))
            qk_sb = [big.tile([128, T], BF16, tag=f"qk{j}", name=f"qk_sb{j}") for j in range(8)]
            v8 = big.tile([128, 8 * HG * 2 * VP], FP8E4, tag="v8", name="v8")
            v8v = v8[:].rearrange("p (pr h t d) -> p pr h t d", pr=8, h=HG, t=2, d=VP)
            vb = big.tile([128, NK * HG * 65], BF16, tag="vb", name="vb")
            vbv = vb[:].rearrange("p (i h d) -> p i h d", i=NK, h=HG, d=65)
            y_sb = [big.tile([128, T], BF16, tag=f"y{m}", name=f"y_sb{m}") for m in range(4)]
            onesr = big.tile([1, 64], F32R, tag="onesr", name="onesr")
            dmask_sb = big.tile([128, 128], BF16, tag="dm", name="dmask_sb")
            ident_sb = big.tile([128, 128], BF16, tag="ident", name="ident_sb")
            xtb = big.tile([128, NC_ * T], BF16, tag="xtb", name="xtb")
            wqkb = big.tile([128, NC_ * 1024], BF16, tag="wqkb", name="wqkb")
            wvb = big.tile([128, NC_ * 512], BF16, tag="wvb", name="wvb")
            wo_sb = big.tile([128, 2 * 4 * 512], BF16, tag="wo", name="wo_sb")
            wov = wo_sb[:].rearrange("p (e j) -> p e j", e=2, j=4 * 512)

            nc.gpsimd.dma_start(onesr[:], ones_row[:])
            nc.gpsimd.dma_start(dmask_sb[:], dmask_d[:])
            nc.gpsimd.dma_start(ident_sb[:], ident_d[:])
            # packed layouts: xt = [p, (tt ct t')], wqk = [p, (jt ct j')]
            xtv = xtb[:].rearrange("p (s c t) -> p s c t", s=NT, c=NC_, t=JW)
            xtdv = xtb_d[:].rearrange("p (s c t) -> p s c t", s=NT, c=NC_, t=JW)
            wqkv = wqkb[:].rearrange("p (g c j) -> p g c j", g=8, c=NC_, j=128)
            wqkdv = wqk_d[:].rearrange("p (g c j) -> p g c j", g=8, c=NC_, j=128)
            # split DMAs, ordered so the first projection chunks start early
            nc.sync.dma_start(wqkv[:, 4], wqkdv[:, 4])
            nc.sync.dma_start(wqkv[:, 0], wqkdv[:, 0])
            nc.sync.dma_start(xtv[:, 0, 0:4], xtdv[:, 0, 0:4])
            nc.gpsimd.dma_start(xtv[:, 0, 4:NC_], xtdv[:, 0, 4:NC_])
            nc.gpsimd.dma_start(wvb[:], wv_d[:])
            for jt in (5, 1, 6, 2, 7, 3):
                nc.gpsimd.dma_start(wqkv[:, jt], wqkdv[:, jt])
            for tt in range(1, NT):
                nc.gpsimd.dma_start(xtv[:, tt], xtdv[:, tt])
            for jt in range(4):
                for et in range(2):
                    nc.gpsimd.dma_start(
                        wov[:, et, 512 * jt:512 * jt + 512],
                        wout_d[128 * jt:128 * jt + 128, 512 * et:512 * et + 512])
            wvv = wvb[:].rearrange("p (c j) -> p c j", c=NC_, j=512)
            for pr in range(8):
                nc.vector.memset(v8v[:, pr, :, :, 64], 1.0)
                nc.vector.memset(v8v[:, pr, :, :, 65:VP], 0.0)
            nc.vector.memset(vbv[:, :, :, 64], 1.0)

            # P buffers
            p8_pool = ctx.enter_context(tc.tile_pool(name="p8", bufs=2))
            pd_pool = ctx.enter_context(tc.tile_pool(name="pd", bufs=2))
            fin_pool = ctx.enter_context(tc.tile_pool(name="fin", bufs=2))
            # PSUM: s 2x2 banks + ya/yb 1 each + aux 2 = 8 banks
            s_psum = ctx.enter_context(tc.tile_pool(name="s_psum", bufs=2, space="PSUM"))
            y_psum = ctx.enter_context(tc.tile_pool(name="y_psum", bufs=1, space="PSUM"))
            aux_psum = ctx.enter_context(tc.tile_pool(name="aux_psum", bufs=2, space="PSUM"))
            o_pool = ctx.enter_context(tc.tile_pool(name="o", bufs=2))

            # ---------- PE work-unit emitters ----------
            def proj_qk(jt, tt):
                def emit():
                    ps = aux_psum.tile([128, JW], F32, tag="aux", name="psaux")
                    for ct in range(NC_):
                        nc.tensor.matmul(
                            ps[:], wqkv[:, jt, ct, :],
                            xtv[:, tt, ct, :],
                            start=(ct == 0), stop=(ct == NC_ - 1))
                    nc.scalar.copy(qk_sb[jt][:, JW * tt:JW * tt + JW], ps[:])
                return emit

            def proj_v(it):
                def emit():
                    ps = aux_psum.tile([128, JW], F32, tag="aux", name="psaux")
                    for ct in range(NC_):
                        nc.tensor.matmul(
                            ps[:], xtv[:, it // 4, ct, 128 * (it % 4):128 * (it % 4) + 128],
                            wvv[:, ct, :],
                            start=(ct == 0), stop=(ct == NC_ - 1))
                    psv = ps[:].rearrange("p (h d) -> p h d", h=HG, d=64)
                    nc.scalar.copy(v8v[:, it // 2, :, it % 2, 0:64], psv)
                    nc.vector.tensor_copy(vbv[:, it, :, 0:64], psv)
                return emit

            def outproj(it, et):
                def emit():
                    ps = aux_psum.tile([128, JW], F32, tag="aux", name="psaux")
                    for jt in range(4):
                        nc.tensor.matmul(
                            ps[:], y_sb[jt][:, 128 * it:128 * it + 128],
                            wov[:, et, 512 * jt:512 * jt + 512],
                            start=(jt == 0), stop=(jt == 3))
                    ot = o_pool.tile([128, 512], F32, tag="ot", name="ot")
                    nc.vector.tensor_copy(ot[:], ps[:])
                    nc.sync.dma_start(
                        out[128 * it:128 * it + 128, 512 * et:512 * et + 512], ot[:])
                return emit

            # ordered filler queue with availability gating
            fillers = []          # list of closures
            ready = []            # parallel list of bools
            drained = [0]         # next index to drain

            def add_fill(fn, is_ready=True):
                fillers.append(fn)
                ready.append(is_ready)
                return len(fillers) - 1

            def drain(n):
                k = 0
                while k < n and drained[0] < len(fillers) and ready[drained[0]]:
                    fillers[drained[0]]()
                    drained[0] += 1
                    k += 1

            def flush_to(idx):
                while drained[0] < idx:
                    assert ready[drained[0]], f"filler {drained[0]} not ready"
                    fillers[drained[0]]()
                    drained[0] += 1

            # build the static filler order
            levels = {}
            for m in range(1, 4):
                add_fill(proj_qk(4 + m, 0))
                add_fill(proj_qk(m, 0))
                levels[(0, m)] = len(fillers)
            op_idx = {}
            # segment for J=1: V tiles 4-7 + projections tt=1
            for it in range(4, 8):
                add_fill(proj_v(it))
            for jt in (4, 5, 6, 7, 0, 1, 2, 3):
                add_fill(proj_qk(jt, 1))
            levels[(1, 0)] = len(fillers)
            # drained during J=1: op(J0), V 8-11, tt2
            op_idx[0] = [add_fill(outproj(it, et), is_ready=False)
                         for it in range(0, 4) for et in range(2)]
            for it in range(8, 12):
                add_fill(proj_v(it))
            for jt in (4, 5, 6, 7, 0, 1, 2, 3):
                add_fill(proj_qk(jt, 2))
            levels[(2, 0)] = len(fillers)
            # drained during J=2: op(J1), tt3
            op_idx[1] = [add_fill(outproj(it, et), is_ready=False)
                         for it in range(4, 8) for et in range(2)]
            for jt in (4, 5, 6, 7, 0, 1, 2, 3):
                add_fill(proj_qk(jt, 3))
            levels[(3, 0)] = len(fillers)
            # drained during J=3: V 12-15 (used by J3's own PV, order-gated), op(J2)
            for it in range(12, 16):
                add_fill(proj_v(it))
            op_idx[2] = [add_fill(outproj(it, et), is_ready=False)
                         for it in range(8, 12) for et in range(2)]
            op_idx[3] = [add_fill(outproj(it, et), is_ready=False)
                         for it in range(12, 16) for et in range(2)]

            # ---------- phase 0: first projections ----------
            proj_qk(4, 0)()   # k of head-pair 0
            proj_qk(0, 0)()   # q of head-pair 0
            for it in range(4):
                proj_v(it)()

            # ---------- main attention loop ----------
            ESC = 0.125
            pending_norm = []

            def emit_s(J, m, i):
                r = i - 4 * J
                lo = 128 * r if r > 0 else 0
                sch = s_psum.tile([128, 2 * JW], F32, tag="s", name="S")
                schv = sch[:].rearrange("p (o q) -> p o q", o=2, q=JW)
                diag = r >= 0
                for oi, off in enumerate((0, 64)):
                    nc.tensor.matmul(
                        schv[:, oi, lo:JW],
                        qk_sb[4 + m][off:off + 64, 128 * i:128 * i + 128],
                        qk_sb[m][off:off + 64, JW * J + lo:JW * J + JW],
                        start=True, stop=not diag,
                        skip_group_check=diag)
                if diag:
                    for oi in range(2):
                        nc.tensor.matmul(
                            schv[:, oi, 128 * r:128 * r + 128],
                            ident_sb[:], dmask_sb[:],
                            start=False, stop=True,
                            skip_group_check=True)
                return schv

            blocks = [(J, m) for J in range(NT) for m in range(4)]
            prefetch = {}
            for bi, (J, m) in enumerate(blocks):
                pace = 1 if J <= 1 else (3 if J == 2 else 4)
                lvl = levels.get((J, m))
                if lvl is not None:
                    flush_to(lvl)
                psy = {0: y_psum.tile([VP, JW], F32, tag="ya", name="psya"),
                       64: y_psum.tile([VP, JW], F32, tag="yb", name="psyb")}
                nki = 4 * J + 4
                p8v = None
                if J > 0:
                    p8 = p8_pool.tile([128, 2 * 6 * 2 * JW], FP8E5, tag="p8")
                    p8v = p8[:].rearrange("p (o pr t q) -> p o pr t q",
                                          o=2, pr=6, t=2, q=JW)
                pd = pd_pool.tile([128, 2 * 4 * JW], BF16, tag="pd")
                pdv = pd[:].rearrange("p (o r q) -> p o r q", o=2, r=4, q=JW)

                first_pv = {0: True, 64: True}
                n_pv = (2 * J) + 4          # DR pairs + diag singles per off
                pv_done = {0: 0, 64: 0}
                pending_pv = []

                def emit_pv():
                    for fn in pending_pv:
                        fn()
                    pending_pv.clear()

                for i in range(nki):
                    r = i - 4 * J
                    lo = 128 * r if r > 0 else 0
                    diag = r >= 0
                    if i == 0 and (J, m) in prefetch:
                        schv = prefetch.pop((J, m))
                    else:
                        schv = emit_s(J, m, i)
                    if diag:
                        nc.scalar.activation(
                            pdv[:, :, r, lo:JW], schv[:, :, lo:JW], EXP, scale=ESC)

                        def mk_diag(i=i, r=r, lo=lo, m=m, psy=psy, pdv=pdv,
                                    first_pv=first_pv, pv_done=pv_done, n_pv=n_pv):
                            def go():
                                for oi, off in enumerate((0, 64)):
                                    h = 2 * m + oi
                                    pv_done[off] += 1
                                    nc.tensor.matmul(
                                        psy[off][0:65, lo:JW],
                                        vbv[:, i, h, :],
                                        pdv[:, oi, r, lo:JW],
                                        start=first_pv[off],
                                        stop=(pv_done[off] == n_pv),
                                        skip_group_check=True)
                                    first_pv[off] = False
                            return go
                        pending_pv.append(mk_diag())
                    else:
                        nc.scalar.activation(
                            p8v[:, :, i // 2, i % 2, :], schv[:, :, :], EXP, scale=ESC)
                        if i % 2 == 1:
                            def mk_pair(i=i, m=m, psy=psy, p8v=p8v,
                                        first_pv=first_pv, pv_done=pv_done, n_pv=n_pv):
                                def go():
                                    for oi, off in enumerate((0, 64)):
                                        h = 2 * m + oi
                                        pv_done[off] += 1
                                        nc.tensor.matmul(
                                            psy[off][:],
                                            v8v[:, i // 2, h, :, :],
                                            p8v[:, oi, i // 2, :, :],
                                            start=first_pv[off],
                                            stop=(pv_done[off] == n_pv),
                                            perf_mode=DR, skip_group_check=True)
                                        first_pv[off] = False
                                return go
                            pending_pv.append(mk_pair())
                    if i == nki - 1 and bi + 1 < len(blocks):
                        # prefetch next block's first S chunk so exp never
                        # stalls across the block boundary
                        Jn, mn = blocks[bi + 1]
                        lvln = levels.get((Jn, mn))
                        if lvln is not None:
                            flush_to(lvln)
                        prefetch[(Jn, mn)] = emit_s(Jn, mn, 0)
                    # filler work covers the exp latency before PV lands
                    if i % pace == 0:
                        drain(1)
                    if i == 1 and pending_norm:
                        for fn in pending_norm:
                            fn()
                        pending_norm.clear()
                    if i >= 1:
                        emit_pv()
                emit_pv()

                def mk_norm(m=m, J=J, psy=psy):
                    def go():
                        for off in (0, 64):
                            rsr = fin_pool.tile([1, JW], F32, tag="rsr", name="rsr")
                            nc.vector.tensor_copy(rsr[:], psy[off][64:65, :])
                            rrow = fin_pool.tile([1, JW], F32, tag="rrow", name="rrow")
                            nc.vector.reciprocal_approx_fast(rrow[:], rsr[:])
                            rec = fin_pool.tile([64, JW], F32, tag="rec", name="rec")
                            nc.gpsimd.partition_broadcast(rec[:], rrow[:])
                            nc.vector.tensor_mul(
                                y_sb[m][off:off + 64, JW * J:JW * J + JW],
                                psy[off][0:64, :], rec[:])
                        if m == 3:
                            # y(J) now complete: release its out-proj fillers
                            for idx in op_idx[J]:
                                ready[idx] = True
                    return go
                pending_norm.append(mk_norm())
                # last block: no next chunk loop to host the deferred normalize
                if (J, m) == blocks[-1]:
                    for fn in pending_norm:
                        fn()
                    pending_norm.clear()
            # tail: remaining out-projection (+ any stragglers)
            flush_to(len(fillers))
    nc.compile()
    return nc


def _host_masks():
    a = np.arange(128, dtype=np.int64)[:, None]
    b = np.arange(128, dtype=np.int64)[None, :]
    return np.where(a <= b, np.float32(0.0), np.float32(MASK_VAL)).astype(BFNP)


def _pack_ct(arr):
    """[1024, n] f32 -> [128, 8*n] bf16 with c = 128*ct + p packing."""
    n = arr.shape[1]
    return np.ascontiguousarray(
        arr.reshape(NC_, 128, n).transpose(1, 0, 2).reshape(128, NC_ * n)
        .astype(BFNP))


def _pack_blocked(arr, blk):
    """[1024, n] f32 -> [128, (n//blk)*8*blk] bf16, free = (slice, ct, blk)."""
    n = arr.shape[1]
    return np.ascontiguousarray(
        arr.reshape(NC_, 128, n // blk, blk).transpose(1, 2, 0, 3)
        .reshape(128, -1).astype(BFNP))


def _make_in_map(core, x, w_qkv, w_out):
    b, g = divmod(core, 2)
    xT = np.ascontiguousarray(x[b].T)
    wqk = np.concatenate(
        [w_qkv[:, 512 * g:512 * g + 512],
         w_qkv[:, 1024 + 512 * g:1024 + 512 * g + 512]], axis=1)
    wv = w_qkv[:, 2048 + 512 * g:2048 + 512 * g + 512]
    wout_s = np.ascontiguousarray(w_out[512 * g:512 * g + 512, :]).astype(BFNP)
    return dict(
        xtb=_pack_blocked(xT, JW),
        wqk=_pack_blocked(wqk, 128),
        wv=_pack_ct(wv),
        wout=wout_s,
        dmask=_host_masks(),
        ident=np.eye(128, dtype=np.float32).astype(BFNP),
        ones_row=np.ones((1, 64), np.float32))


def kernel(x, w_qkv, w_out):
    x = np.ascontiguousarray(x, dtype=np.float32)
    w_qkv = np.ascontiguousarray(w_qkv, dtype=np.float32)
    w_out = np.ascontiguousarray(w_out, dtype=np.float32)

    if "nc" not in _cache:
        _cache["nc"] = _build()
    nc = _cache["nc"]

    in_maps = [_make_in_map(core, x, w_qkv, w_out) for core in range(8)]

    res = run_bass_kernel_spmd(nc, in_maps, core_ids=list(range(8)))
    out = np.empty((B, T, C), np.float32)
    for b in range(B):
        out[b] = res.results[2 * b]["out"] + res.results[2 * b + 1]["out"]
    return out


# revision 21
# speedup vs baseline: 1.3215x; 1.0113x over previous
"""Causal self-attention (B=4, T=2048, C=1024, H=16, Dh=64) on 8 trn2 NeuronCores.

Sharding: core = 2*b + g  (b = batch 0..3, g = head-group 0..1, 8 heads each).
Each core computes its batch's QKV projection for its 8 heads, causal
attention, and a partial out-projection; host sums the two head-group
partials per batch (the tensor-parallel "all-reduce").

v3 design (per core), single software-pipelined loop:
  - All projections bf16 (fp8 there costs ~3-5% output error).
  - q^T/k^T bf16 [j, t]; S^T[tk, tq] per head-pair computed into fp32-psum
    chunks [128, 2, 512] (both heads) -> ONE fused exp (ACT) per tk tile.
  - exp writes P: diagonal-straddling tiles -> bf16; strictly-causal tiles ->
    fp8 e5m2 (no max-subtraction; e5m2 spans e^-14..e^11; softmax averaging
    damps the 2-bit mantissa noise).
  - PV: off-diag via fp8 DoubleRow over tk-tile pairs (V e4m3 lhsT, d-dim
    padded to 80 for the 16B DoubleRow stride rule, ones col 64 = rowsum);
    diag tiles bf16. PV emission lags exp by one chunk so the PE never
    blocks on ACT.
  - Projection / V / out-projection matmul chunks are interleaved between
    attention chunks from an ordered filler queue, keeping the PE
    continuously busy (full 2.4 GHz pstate) across the whole kernel.
  - reciprocal + K=1 ones matmul broadcasts 1/rowsum; DVE mul -> y^T bf16.
  - out-projection bf16, one tq-block behind attention.
"""

import sys

for _p in ("/opt/trn_rl_repo", "/opt/pypackages"):
    if _p not in sys.path:
        sys.path.append(_p)

import numpy as np
import ml_dtypes
from contextlib import ExitStack

import concourse.bass as bass
import concourse.tile as tile
from concourse import bacc, mybir
from concourse.bass_utils import run_bass_kernel_spmd

B, T, C = 4, 2048, 1024
H, DH = 16, 64
HG = 8          # heads per core
JW = 512        # tq tile width
NT = T // JW    # 4 tq tiles
NK = T // 128   # 16 tk tiles
NC_ = C // 128  # 8 contraction tiles
VP = 80         # padded V free dim (16B-aligned for DoubleRow)
MASK_VAL = -1.0e8
F32 = mybir.dt.float32
F32R = mybir.dt.float32r
BF16 = mybir.dt.bfloat16
FP8E4 = mybir.dt.float8e4
FP8E5 = mybir.dt.float8e5
EXP = mybir.ActivationFunctionType.Exp
DR = mybir.MatmulPerfMode.DoubleRow

E4NP = ml_dtypes.float8_e4m3
E5NP = ml_dtypes.float8_e5m2
BFNP = ml_dtypes.bfloat16

_cache = {}


def _build():
    nc = bacc.Bacc("TRN2", target_bir_lowering=False, debug=False, num_devices=8)
    xtb_d = nc.dram_tensor("xtb", [128, NC_ * T], BF16, kind="ExternalInput").ap()
    wqk_d = nc.dram_tensor("wqk", [128, NC_ * 1024], BF16, kind="ExternalInput").ap()
    wv_d = nc.dram_tensor("wv", [128, NC_ * 512], BF16, kind="ExternalInput").ap()
    wout_d = nc.dram_tensor("wout", [512, C], BF16, kind="ExternalInput").ap()
    dmask_d = nc.dram_tensor("dmask", [128, 128], BF16, kind="ExternalInput").ap()
    ident_d = nc.dram_tensor("ident", [128, 128], BF16, kind="ExternalInput").ap()
    ones_row = nc.dram_tensor("ones_row", [1, 64], F32R, kind="ExternalInput").ap()
    out = nc.dram_tensor("out", [T, C], F32, kind="ExternalOutput").ap()

    with tile.TileContext(nc) as tc:
        with ExitStack() as ctx:
            ctx.enter_context(nc.allow_low_precision(reason="fp8/bf16 mixed precision intended"))
            # ---- persistent SBUF ----
            big = ctx.enter_context(tc.tile_pool(name="big", bufs=1))
            qk_sb = [big.tile([128, T], BF16, tag=f"qk{j}", name=f"qk_sb{j}") for j in range(8)]
            v8 = big.tile([128, 8 * HG * 2 * VP], FP8E4, tag="v8", name="v8")
            v8v = v8[:].rearrange("p (pr h t d) -> p pr h t d", pr=8, h=HG, t=2, d=VP)
            vb = big.tile([128, NK * HG * 65], BF16, tag="vb", name="vb")
            vbv = vb[:].rearrange("p (i h d) -> p i h d", i=NK, h=HG, d=65)
            y_sb = [big.tile([128, T], BF16, tag=f"y{m}", name=f"y_sb{m}") for m in range(4)]
            onesr = big.tile([1, 64], F32R, tag="onesr", name="onesr")
            dmask_sb = big.tile([128, 128], BF16, tag="dm", name="dmask_sb")
            ident_sb = big.tile([128, 128], BF16, tag="ident", name="ident_sb")
            xtb = big.tile([128, NC_ * T], BF16, tag="xtb", name="xtb")
            wqkb = big.tile([128, NC_ * 1024], BF16, tag="wqkb", name="wqkb")
            wvb = big.tile([128, NC_ * 512], BF16, tag="wvb", name="wvb")
            wo_sb = big.tile([128, 2 * 4 * 512], BF16, tag="wo", name="wo_sb")
            wov = wo_sb[:].rearrange("p (e j) -> p e j", e=2, j=4 * 512)


            # packed layouts: xt = [p, (tt ct t')], wqk = [p, (jt ct j')]
            xtv = xtb[:].rearrange("p (s c t) -> p s c t", s=NT, c=NC_, t=JW)
            xtdv = xtb_d[:].rearrange("p (s c t) -> p s c t", s=NT, c=NC_, t=JW)
            wqkv = wqkb[:].rearrange("p (g c j) -> p g c j", g=8, c=NC_, j=128)
            wqkdv = wqk_d[:].rearrange("p (g c j) -> p g c j", g=8, c=NC_, j=128)
            # split DMAs, ordered so the first projection chunks start early
            nc.sync.dma_start(xtv[:, 0, 0:4], xtdv[:, 0, 0:4])
            nc.gpsimd.dma_start(xtv[:, 0, 4:NC_], xtdv[:, 0, 4:NC_])
            nc.sync.dma_start(wqkv[:, 4], wqkdv[:, 4])
            nc.gpsimd.dma_start(wqkv[:, 0], wqkdv[:, 0])
            nc.sync.dma_start(dmask_sb[:], dmask_d[:])
            nc.sync.dma_start(ident_sb[:], ident_d[:])
            nc.sync.dma_start(onesr[:], ones_row[:])
            nc.gpsimd.dma_start(wvb[:], wv_d[:])
            for jt in (5, 1, 6, 2, 7, 3):
                nc.gpsimd.dma_start(wqkv[:, jt], wqkdv[:, jt])
            for tt in range(1, NT):
                nc.gpsimd.dma_start(xtv[:, tt], xtdv[:, tt])
            for jt in range(4):
                for et in range(2):
                    nc.gpsimd.dma_start(
                        wov[:, et, 512 * jt:512 * jt + 512],
                        wout_d[128 * jt:128 * jt + 128, 512 * et:512 * et + 512])
            wvv = wvb[:].rearrange("p (c j) -> p c j", c=NC_, j=512)
            for pr in range(8):
                nc.vector.memset(v8v[:, pr, :, :, 64], 1.0)
                nc.vector.memset(v8v[:, pr, :, :, 65:VP], 0.0)
            nc.vector.memset(vbv[:, :, :, 64], 1.0)

            # P buffers
            p8_pool = ctx.enter_context(tc.tile_pool(name="p8", bufs=2))
            pd_pool = ctx.enter_context(tc.tile_pool(name="pd", bufs=2))
            fin_pool = ctx.enter_context(tc.tile_pool(name="fin", bufs=2))
            # PSUM: s 2x2 banks + ya/yb 1 each + aux 2 = 8 banks
            s_psum = ctx.enter_context(tc.tile_pool(name="s_psum", bufs=2, space="PSUM"))
            y_psum = ctx.enter_context(tc.tile_pool(name="y_psum", bufs=1, space="PSUM"))
            aux_psum = ctx.enter_context(tc.tile_pool(name="aux_psum", bufs=2, space="PSUM"))
            o_pool = ctx.enter_context(tc.tile_pool(name="o", bufs=2))

            # ---------- PE work-unit emitters ----------
            def proj_qk(jt, tt):
                def emit():
                    ps = aux_psum.tile([128, JW], F32, tag="aux", name="psaux")
                    for ct in range(NC_):
                        nc.tensor.matmul(
                            ps[:], wqkv[:, jt, ct, :],
                            xtv[:, tt, ct, :],
                            start=(ct == 0), stop=(ct == NC_ - 1))
                    nc.scalar.copy(qk_sb[jt][:, JW * tt:JW * tt + JW], ps[:])
                return emit

            def proj_v(it):
                def emit():
                    ps = aux_psum.tile([128, JW], F32, tag="aux", name="psaux")
                    for ct in range(NC_):
                        nc.tensor.matmul(
                            ps[:], xtv[:, it // 4, ct, 128 * (it % 4):128 * (it % 4) + 128],
                            wvv[:, ct, :],
                            start=(ct == 0), stop=(ct == NC_ - 1))
                    psv = ps[:].rearrange("p (h d) -> p h d", h=HG, d=64)
                    nc.scalar.copy(v8v[:, it // 2, :, it % 2, 0:64], psv)
                    nc.vector.tensor_copy(vbv[:, it, :, 0:64], psv)
                return emit

            def outproj(it, et):
                def emit():
                    ps = aux_psum.tile([128, JW], F32, tag="aux", name="psaux")
                    for jt in range(4):
                        nc.tensor.matmul(
                            ps[:], y_sb[jt][:, 128 * it:128 * it + 128],
                            wov[:, et, 512 * jt:512 * jt + 512],
                            start=(jt == 0), stop=(jt == 3))
                    ot = o_pool.tile([128, 512], F32, tag="ot", name="ot")
                    nc.vector.tensor_copy(ot[:], ps[:])
                    nc.sync.dma_start(
                        out[128 * it:128 * it + 128, 512 * et:512 * et + 512], ot[:])
                return emit

            # ordered filler queue with availability gating
            fillers = []          # list of closures
            ready = []            # parallel list of bools
            drained = [0]         # next index to drain

            def add_fill(fn, is_ready=True):
                fillers.append(fn)
                ready.append(is_ready)
                return len(fillers) - 1

            def drain(n):
                k = 0
                while k < n and drained[0] < len(fillers) and ready[drained[0]]:
                    fillers[drained[0]]()
                    drained[0] += 1
                    k += 1

            def flush_to(idx):
                while drained[0] < idx:
                    assert ready[drained[0]], f"filler {drained[0]} not ready"
                    fillers[drained[0]]()
                    drained[0] += 1

            # build the static filler order
            levels = {}
            for m in range(1, 4):
                add_fill(proj_qk(4 + m, 0))
                add_fill(proj_qk(m, 0))
                levels[(0, m)] = len(fillers)
            op_idx = {}
            # segment for J=1: V tiles 4-7 + projections tt=1
            for it in range(4, 8):
                add_fill(proj_v(it))
            for jt in (4, 5, 6, 7, 0, 1, 2, 3):
                add_fill(proj_qk(jt, 1))
            levels[(1, 0)] = len(fillers)
            # drained during J=1: op(J0), V 8-11, tt2
            op_idx[0] = [add_fill(outproj(it, et), is_ready=False)
                         for it in range(0, 4) for et in range(2)]
            for it in range(8, 12):
                add_fill(proj_v(it))
            for jt in (4, 5, 6, 7, 0, 1, 2, 3):
                add_fill(proj_qk(jt, 2))
            levels[(2, 0)] = len(fillers)
            # drained during J=2: op(J1), tt3
            op_idx[1] = [add_fill(outproj(it, et), is_ready=False)
                         for it in range(4, 8) for et in range(2)]
            for jt in (4, 5, 6, 7, 0, 1, 2, 3):
                add_fill(proj_qk(jt, 3))
            levels[(3, 0)] = len(fillers)
            # drained during J=3: V 12-15 (used by J3's own PV, order-gated), op(J2)
            for it in range(12, 16):
                add_fill(proj_v(it))
            op_idx[2] = [add_fill(outproj(it, et), is_ready=False)
                         for it in range(8, 12) for et in range(2)]
            op_idx[3] = [add_fill(outproj(it, et), is_ready=False)
                         for it in range(12, 16) for et in range(2)]

            # ---------- phase 0: first projections ----------
            proj_qk(4, 0)()   # k of head-pair 0
            proj_qk(0, 0)()   # q of head-pair 0
            for it in range(4):
                proj_v(it)()

            # ---------- main attention loop ----------
            ESC = 0.125
            pending_norm = []

            def emit_s(J, m, i):
                r = i - 4 * J
                lo = 128 * r if r > 0 else 0
                sch = s_psum.tile([128, 2 * JW], F32, tag="s", name="S")
                schv = sch[:].rearrange("p (o q) -> p o q", o=2, q=JW)
                diag = r >= 0
                pe_mask = diag and J >= 2
                for oi, off in enumerate((0, 64)):
                    nc.tensor.matmul(
                        schv[:, oi, lo:JW],
                        qk_sb[4 + m][off:off + 64, 128 * i:128 * i + 128],
                        qk_sb[m][off:off + 64, JW * J + lo:JW * J + JW],
                        start=True, stop=not pe_mask,
                        skip_group_check=pe_mask)
                if pe_mask:
                    for oi in range(2):
                        nc.tensor.matmul(
                            schv[:, oi, 128 * r:128 * r + 128],
                            ident_sb[:], dmask_sb[:],
                            start=False, stop=True,
                            skip_group_check=True)
                elif diag:
                    for oi in range(2):
                        nc.vector.tensor_add(
                            schv[:, oi, 128 * r:128 * r + 128],
                            schv[:, oi, 128 * r:128 * r + 128],
                            dmask_sb[:])
                return schv

            blocks = [(J, m) for J in range(NT) for m in range(4)]
            prefetch = {}
            for bi, (J, m) in enumerate(blocks):
                pace = 1 if J <= 1 else (3 if J == 2 else 4)
                lvl = levels.get((J, m))
                if lvl is not None:
                    flush_to(lvl)
                psy = {0: y_psum.tile([VP, JW], F32, tag="ya", name="psya"),
                       64: y_psum.tile([VP, JW], F32, tag="yb", name="psyb")}
                nki = 4 * J + 4
                p8v = None
                if J > 0:
                    p8 = p8_pool.tile([128, 2 * 6 * 2 * JW], FP8E5, tag="p8")
                    p8v = p8[:].rearrange("p (o pr t q) -> p o pr t q",
                                          o=2, pr=6, t=2, q=JW)
                pd = pd_pool.tile([128, 2 * 4 * JW], BF16, tag="pd")
                pdv = pd[:].rearrange("p (o r q) -> p o r q", o=2, r=4, q=JW)

                first_pv = {0: True, 64: True}
                n_pv = (2 * J) + 4          # DR pairs + diag singles per off
                pv_done = {0: 0, 64: 0}
                pending_pv = []

                def emit_pv():
                    for fn in pending_pv:
                        fn()
                    pending_pv.clear()

                for i in range(nki):
                    r = i - 4 * J
                    lo = 128 * r if r > 0 else 0
                    diag = r >= 0
                    if i == 0 and (J, m) in prefetch:
                        schv = prefetch.pop((J, m))
                    else:
                        schv = emit_s(J, m, i)
                    if diag:
                        nc.scalar.activation(
                            pdv[:, :, r, lo:JW], schv[:, :, lo:JW], EXP, scale=ESC)

                        def mk_diag(i=i, r=r, lo=lo, m=m, psy=psy, pdv=pdv,
                                    first_pv=first_pv, pv_done=pv_done, n_pv=n_pv):
                            def go():
                                for oi, off in enumerate((0, 64)):
                                    h = 2 * m + oi
                                    pv_done[off] += 1
                                    nc.tensor.matmul(
                                        psy[off][0:65, lo:JW],
                                        vbv[:, i, h, :],
                                        pdv[:, oi, r, lo:JW],
                                        start=first_pv[off],
                                        stop=(pv_done[off] == n_pv),
                                        skip_group_check=True)
                                    first_pv[off] = False
                            return go
                        pending_pv.append(mk_diag())
                    else:
                        nc.scalar.activation(
                            p8v[:, :, i // 2, i % 2, :], schv[:, :, :], EXP, scale=ESC)
                        if i % 2 == 1:
                            def mk_pair(i=i, m=m, psy=psy, p8v=p8v,
                                        first_pv=first_pv, pv_done=pv_done, n_pv=n_pv):
                                def go():
                                    for oi, off in enumerate((0, 64)):
                                        h = 2 * m + oi
                                        pv_done[off] += 1
                                        nc.tensor.matmul(
                                            psy[off][:],
                                            v8v[:, i // 2, h, :, :],
                                            p8v[:, oi, i // 2, :, :],
                                            start=first_pv[off],
                                            stop=(pv_done[off] == n_pv),
                                            perf_mode=DR, skip_group_check=True)
                                        first_pv[off] = False
                                return go
                            pending_pv.append(mk_pair())
                    if i == nki - 1 and bi + 1 < len(blocks):
                        # prefetch next block's first S chunk so exp never
                        # stalls across the block boundary
                        Jn, mn = blocks[bi + 1]
                        lvln = levels.get((Jn, mn))
                        if lvln is not None:
                            flush_to(lvln)
                        prefetch[(Jn, mn)] = emit_s(Jn, mn, 0)
                    # filler work covers the exp latency before PV lands
                    if i % pace == 0:
                        drain(1)
                    if i == 1 and pending_norm:
                        for fn in pending_norm:
                            fn()
                        pending_norm.clear()
                    if i >= 1:
                        emit_pv()
                emit_pv()

                def mk_norm(m=m, J=J, psy=psy):
                    def go():
                        for off in (0, 64):
                            rsr = fin_pool.tile([1, JW], F32, tag="rsr", name="rsr")
                            nc.vector.tensor_copy(rsr[:], psy[off][64:65, :])
                            rrow = fin_pool.tile([1, JW], F32, tag="rrow", name="rrow")
                            nc.vector.reciprocal_approx_fast(rrow[:], rsr[:])
                            rec = fin_pool.tile([64, JW], F32, tag="rec", name="rec")
                            nc.gpsimd.partition_broadcast(rec[:], rrow[:])
                            nc.vector.tensor_mul(
                                y_sb[m][off:off + 64, JW * J:JW * J + JW],
                                psy[off][0:64, :], rec[:])
                        if m == 3:
                            # y(J) now complete: release its out-proj fillers
                            for idx in op_idx[J]:
                                ready[idx] = True
                    return go
                pending_norm.append(mk_norm())
                # last block: no next chunk loop to host the deferred normalize
                if (J, m) == blocks[-1]:
                    for fn in pending_norm:
                        fn()
                    pending_norm.clear()
            # tail: remaining out-projection (+ any stragglers)
            flush_to(len(fillers))
    nc.compile()
    return nc


def _host_masks():
    a = np.arange(128, dtype=np.int64)[:, None]
    b = np.arange(128, dtype=np.int64)[None, :]
    return np.where(a <= b, np.float32(0.0), np.float32(MASK_VAL)).astype(BFNP)


def _pack_ct(arr):
    """[1024, n] f32 -> [128, 8*n] bf16 with c = 128*ct + p packing."""
    n = arr.shape[1]
    return np.ascontiguousarray(
        arr.reshape(NC_, 128, n).transpose(1, 0, 2).reshape(128, NC_ * n)
        .astype(BFNP))


def _pack_blocked(arr, blk):
    """[1024, n] f32 -> [128, (n//blk)*8*blk] bf16, free = (slice, ct, blk)."""
    n = arr.shape[1]
    return np.ascontiguousarray(
        arr.reshape(NC_, 128, n // blk, blk).transpose(1, 2, 0, 3)
        .reshape(128, -1).astype(BFNP))


def _make_in_map(core, x, w_qkv, w_out):
    b, g = divmod(core, 2)
    xT = np.ascontiguousarray(x[b].T)
    wqk = np.concatenate(
        [w_qkv[:, 512 * g:512 * g + 512],
         w_qkv[:, 1024 + 512 * g:1024 + 512 * g + 512]], axis=1)
    wv = w_qkv[:, 2048 + 512 * g:2048 + 512 * g + 512]
    wout_s = np.ascontiguousarray(w_out[512 * g:512 * g + 512, :]).astype(BFNP)
    return dict(
        xtb=_pack_blocked(xT, JW),
        wqk=_pack_blocked(wqk, 128),
        wv=_pack_ct(wv),
        wout=wout_s,
        dmask=_host_masks(),
        ident=np.eye(128, dtype=np.float32).astype(BFNP),
        ones_row=np.ones((1, 64), np.float32))


def kernel(x, w_qkv, w_out):
    x = np.ascontiguousarray(x, dtype=np.float32)
    w_qkv = np.ascontiguousarray(w_qkv, dtype=np.float32)
    w_out = np.ascontiguousarray(w_out, dtype=np.float32)

    if "nc" not in _cache:
        _cache["nc"] = _build()
    nc = _cache["nc"]

    in_maps = [_make_in_map(core, x, w_qkv, w_out) for core in range(8)]

    res = run_bass_kernel_spmd(nc, in_maps, core_ids=list(range(8)))
    out = np.empty((B, T, C), np.float32)
    for b in range(B):
        out[b] = res.results[2 * b]["out"] + res.results[2 * b + 1]["out"]
    return out
